# revision 1
# baseline (speedup 1.0000x reference)
"""Trainium2 Bass kernel for nn_Discriminator (DCRNN-style GRU discriminator).

Math restructure (exactly equivalent to the reference):
  dconv(xh, A, W, b) with xh=[x, h] splits into x-terms and h-terms:
    out_g = x W0x + (A x) W1x + (A^2 x) W2x        (precomputed per block, all T at once)
          + h W0h + (A h) W1h + (A^2 h) W2h + bg   (per step)
  A^2 is precomputed on host; A-side matmuls run in bf16 (error ~1e-5),
  feature-contraction (gate) matmuls run in fp32r (tf32-class), elementwise fp32.

Sharding: data-parallel over batch B=4 (cores 4-7 duplicate batches 0-3).
Each core runs both GRU blocks for its batch; host does the final tiny
pred = H[:,-1] @ W_sn + b_out and the mean, in float64.
"""
import numpy as np
import ml_dtypes

import concourse.bass as bass
import concourse.mybir as mybir
import concourse.tile as tile
from concourse import bacc
from concourse import bass_utils
from concourse.masks import make_identity

FP32 = mybir.dt.float32
FP32R = mybir.dt.float32r
BF16 = mybir.dt.bfloat16
AF = mybir.ActivationFunctionType

B, T, N, DIN, DH, K, NBLK = 4, 8, 2048, 64, 64, 3, 2
NC = N // 128            # 16 node chunks (full graph)
NO = 1024                # nodes owned per core
NCO = NO // 128          # 8 owned chunks
NJ = NO // 512           # 2 big column chunks over owned nodes
G = 2 * DH               # 128 gate width


def build_kernel(repeat=1, debug=False, dbg_blk=0, dbg_t=1, trace_sim=False):
    nc = bacc.Bacc(None, target_bir_lowering=False)

    # ---------------- I/O ----------------
    # A transposed / A^2 transposed, bf16 [N, N] (row m = source node)
    AT_d = nc.dram_tensor("AT", [N, NO], BF16, kind="ExternalInput")
    A2T_d = nc.dram_tensor("A2T", [N, NO], BF16, kind="ExternalInput")
    # X node-major stacked features, chunk layout [128, NC*T*DIN] bf16,
    # col c*512 + t*64 + f  <->  X[t, c*128+p, f]
    XF_d = nc.dram_tensor("XF", [128, T * NC * DIN], BF16, kind="ExternalInput")
    # X feat-major [T*DIN, N] fp32 (row t*64+f, col n)
    XT_d = nc.dram_tensor("XT", [T * DIN, NO], BF16, kind="ExternalInput")
    # weights (host spectral-normalized, split, padded):
    # g-path: WGH [NBLK, 65, 128] (rows 0:64 = W0h', row 64 = bg), fp32r
    WGH_d = nc.dram_tensor("WGH", [NBLK, DH + 1, G], BF16, kind="ExternalInput")
    WGH12_d = nc.dram_tensor("WGH12", [NBLK, 2 * DH, G], BF16, kind="ExternalInput")
    WGX0_d = nc.dram_tensor("WGX0", [NBLK, DIN, G], BF16, kind="ExternalInput")
    WGX12_d = nc.dram_tensor("WGX12", [NBLK, 2 * DIN, G], BF16, kind="ExternalInput")
    # c-path
    WCH_d = nc.dram_tensor("WCH", [NBLK, DH + 1, DH], BF16, kind="ExternalInput")
    WCH12_d = nc.dram_tensor("WCH12", [NBLK, 2 * DH, DH], BF16, kind="ExternalInput")
    WCX0_d = nc.dram_tensor("WCX0", [NBLK, DIN, DH], BF16, kind="ExternalInput")
    WCX12_d = nc.dram_tensor("WCX12", [NBLK, 2 * DIN, DH], BF16, kind="ExternalInput")

    HOUT_d = nc.dram_tensor("HOUT", [128, NCO * DH], FP32, kind="ExternalOutput")
    assert not debug, "v2 has no debug dumps"

    with tile.TileContext(nc, trace_sim=trace_sim) as tc:
        with (
            tc.tile_pool(name="big", bufs=1) as big,          # A matrices, persistent
            tc.tile_pool(name="wpool", bufs=1) as wpool,      # weights, identity
            tc.tile_pool(name="state", bufs=2) as state,      # h, h_bf
            tc.tile_pool(name="work", bufs=1) as work,        # hT/rhT/S12/g/rh/cc
            tc.tile_pool(name="stream", bufs=2) as stream,    # XT_t, P12_t, lhsT_mp
            tc.tile_pool(name="scr", bufs=1) as scr,          # elementwise temps
            tc.tile_pool(name="ptr", bufs=2, space="PSUM") as ptr,  # transpose psum
            tc.tile_pool(name="ps12", bufs=2, space="PSUM") as ps12,  # A-mult + precompute psum
            tc.tile_pool(name="pg", bufs=2, space="PSUM") as pg,    # gate psum
            tc.tile_pool(name="dram", bufs=1, space="DRAM") as dram,
        ):
            # ---------- persistent SBUF ----------
            AT_s = big.tile([128, NC * NO], BF16)    # [p, c*NO + x(own)]
            A2T_s = big.tile([128, NC * NO], BF16)
            for src_d, dst in ((AT_d, AT_s), (A2T_d, A2T_s)):
                # dst[p, c*NO + x] = src[c*128 + p, x]
                nc.sync.dma_start(
                    dst[:].rearrange("p (c x) -> p c x", c=NC),
                    src_d[:].rearrange("(c p) x -> p c x", c=NC),
                )

            ident = wpool.tile([128, 128], FP32)
            make_identity(nc, ident[:])
            ident_bf = wpool.tile([128, 128], BF16)
            nc.vector.tensor_copy(ident_bf[:], ident[:])

            # weights to SBUF (one tile per block; SBUF dim0 = partitions)
            def wtiles(dram_t, p, f, dt, nm):
                ts = []
                for blk in range(NBLK):
                    tl = wpool.tile([p, f], dt, name=f"{nm}{blk}", tag=f"{nm}{blk}")
                    nc.sync.dma_start(tl[:], dram_t[blk])
                    ts.append(tl)
                return ts
            wgh = wtiles(WGH_d, DH + 1, G, BF16, "wgh")
            wgh12 = wtiles(WGH12_d, 2 * DH, G, BF16, "wgh12")
            wgx0 = wtiles(WGX0_d, DIN, G, BF16, "wgx0")
            wgx12 = wtiles(WGX12_d, 2 * DIN, G, BF16, "wgx12")
            wch = wtiles(WCH_d, DH + 1, DH, BF16, "wch")
            wch12 = wtiles(WCH12_d, 2 * DH, DH, BF16, "wch12")
            wcx0 = wtiles(WCX0_d, DIN, DH, BF16, "wcx0")
            wcx12 = wtiles(WCX12_d, 2 * DIN, DH, BF16, "wcx12")

            # hT / rhT tiles with a persistent ones-row (row 64) for the bias
            hT = work.tile([DH + 1, NO], BF16)
            ones32 = wpool.tile([1, 512], FP32)
            nc.gpsimd.memset(ones32[:], 1.0)
            for q in range(2):
                nc.vector.tensor_copy(hT[DH:DH + 1, q * 512:(q + 1) * 512], ones32[:])
            rhT = hT

            # staging DRAM for P1^T/P2^T (bf16) and block-2 inputs
            P1T_dr = dram.tile([T * DIN, NO], BF16)
            P2T_dr = dram.tile([T * DIN, NO], BF16)
            H1T_dr = dram.tile([T * DH, NO], BF16)
            H1G_dr = dram.tile([T * N, DH], BF16)
            AGIN_h = dram.tile([NO, DH], BF16)
            AGIN_rh = dram.tile([NO, DH], BF16)
            RHG_a = dram.tile([N, DH], BF16)
            RHG_b = dram.tile([N, DH], BF16)
            HG2_a = dram.tile([N, DH], BF16)
            HG2_b = dram.tile([N, DH], BF16)
            RG = [[b, b + 4] for b in range(4)]

            def dump_p(blk):
                pass

            def load_lhsT_xf(lhsT, mp):
                # lhsT[p, c*128 + tt*64 + f] = XF[p, (2*mp+tt)*1024 + c*64 + f]
                for tt in range(2):
                    nc.sync.dma_start(
                        lhsT[:].rearrange("p (c j) -> p c j", c=NC)[:, :, tt * 64:(tt + 1) * 64],
                        XF_d[:, (2 * mp + tt) * (NC * DIN):(2 * mp + tt + 1) * (NC * DIN)]
                            .rearrange("p (c f) -> p c f", c=NC),
                    )

            def load_lhsT_h1g(lhsT, mp):
                # lhsT[p, c*128 + tt*64 + f] = H1G[(2*mp+tt)*N + c*128 + p, f]
                for tt in range(2):
                    t_ = 2 * mp + tt
                    nc.sync.dma_start(
                        lhsT[:].rearrange("p (c j) -> p c j", c=NC)[:, :, tt * 64:(tt + 1) * 64],
                        H1G_dr[t_ * N:(t_ + 1) * N, :].rearrange("(c p) f -> p c f", c=NC),
                    )

            def precompute(blk, loader):
                """P1^T = (A @ Xf)^T, P2^T = (A^2 @ Xf)^T (own cols) -> DRAM bf16."""
                for mp in range(4):  # M-pass: rows mp*128..mp*128+127 of P^T
                    lhsT = stream.tile([128, NC * 128], BF16, tag="p12", name="lhsT")
                    loader(lhsT, mp)
                    for src, pdst in ((AT_s, P1T_dr), (A2T_s, P2T_dr)):
                        for j in range(NJ):
                            ps = ps12.tile([128, 512], FP32, tag="s12p", name="pp")
                            for c in range(NC):
                                nc.tensor.matmul(
                                    ps[:],
                                    lhsT[:, c * 128:(c + 1) * 128],
                                    src[:, c * NO + j * 512: c * NO + j * 512 + 512],
                                    start=(c == 0), stop=(c == NC - 1),
                                )
                            st = scr.tile([128, 512], BF16, tag="rhbf", name="pstg")
                            nc.vector.tensor_copy(st[:], ps[:])
                            nc.sync.dma_start(
                                pdst[mp * 128:(mp + 1) * 128, j * 512:(j + 1) * 512], st[:])

            def transpose_to(dst_T, src_bf, nrows):
                """PE-transpose bf16 src [128, NCO*nrows] (chunk-major) -> dst_T [nrows, NO]."""
                for q in range(2):
                    pt = ptr.tile([nrows, 512], BF16, tag="ptr", name="ptr")
                    for ci in range(4):
                        c = q * 4 + ci
                        nc.tensor.transpose(
                            pt[:, ci * 128:(ci + 1) * 128],
                            src_bf[:, c * nrows:(c + 1) * nrows],
                            ident_bf[:],
                        )
                    nc.vector.tensor_copy(dst_T[0:nrows, q * 512:(q + 1) * 512], pt[:])

            def gru_block(blk, xt_dram, p1_dram, p2_dram, store_h1):
                # initial state h = 0 (own nodes)
                h = state.tile([128, NCO * DH], FP32, tag="h", name="h")
                nc.gpsimd.memset(h[:], 0.0)

                def gather(src_bf_own, gout_ap, agin):
                    """AllGather own node-major rows -> full [N, DH] bf16 dram."""
                    nc.sync.dma_start(
                        agin[:].rearrange("(ch p) f -> p ch f", ch=NCO),
                        src_bf_own[:].rearrange("p (ch f) -> p ch f", ch=NCO),
                    )
                    nc.gpsimd.collective_compute(
                        "AllGather", mybir.AluOpType.bypass,
                        ins=[agin[:]], outs=[gout_ap],
                        replica_groups=RG,
                    )

                def to_sbuf_full(gout_ap):
                    hf = stream.tile([128, NC * DH], BF16, tag="hfull", name="hfull")
                    nc.sync.dma_start(
                        hf[:].rearrange("p (c f) -> p c f", c=NC),
                        gout_ap.rearrange("(c p) f -> p c f", c=NC),
                    )
                    return hf

                def amult(hf, tag):
                    """S12^T (own cols) from full node-major bf16 lhsT."""
                    s12_ = work.tile([128, NO], BF16, tag="s12", name=tag)
                    for j in range(NJ):
                        ps = ps12.tile([128, 512], FP32, tag="s12p", name="s12p" + tag)
                        for c in range(NC):
                            lh = hf[:, c * DH:(c + 1) * DH]
                            nc.tensor.matmul(
                                ps[0:DH, :], lh,
                                AT_s[:, c * NO + j * 512: c * NO + j * 512 + 512],
                                start=(c == 0), stop=(c == NC - 1))
                            nc.tensor.matmul(
                                ps[DH:128, :], lh,
                                A2T_s[:, c * NO + j * 512: c * NO + j * 512 + 512],
                                start=(c == 0), stop=(c == NC - 1))
                        nc.vector.tensor_copy(s12_[:, j * 512:(j + 1) * 512], ps[:])
                    return s12_

                for t in range(T):
                    # --- stream x-term slices (own nodes) ---
                    xt_t = stream.tile([DIN, NO], BF16, tag="xt", name="xt")
                    nc.sync.dma_start(xt_t[:], xt_dram[t * DIN:(t + 1) * DIN, :])
                    p12_t = stream.tile([128, NO], BF16, tag="p12", name="p12")
                    nc.sync.dma_start(p12_t[0:DIN, :], p1_dram[t * DIN:(t + 1) * DIN, :])
                    nc.sync.dma_start(p12_t[DIN:128, :], p2_dram[t * DIN:(t + 1) * DIN, :])

                    # --- g path ---
                    if t > 0:
                        # gather current h (result of step t-1) across the pair;
                        # issue the collective before local transposes
                        hbf = scr.tile([128, NCO * DH], BF16, tag="hbfg", name="hbf")
                        nc.vector.tensor_copy(hbf[:], h[:])
                        if store_h1:
                            gout = H1G_dr[(t - 1) * N:t * N, :]
                        else:
                            gout = (HG2_a if t % 2 else HG2_b)[:]
                        gather(hbf, gout, AGIN_h)
                        transpose_to(hT, hbf, DH)
                        if store_h1:
                            nc.sync.dma_start(H1T_dr[(t - 1) * DH:t * DH, :], hT[0:DH, :])
                        hf = to_sbuf_full(gout)
                        s12 = amult(hf, "s12g")

                    g = work.tile([128, NCO * G], FP32, tag="g", name="g")
                    for cg in range(2):  # groups of 4 node-chunks -> one psum bank
                        psg = pg.tile([128, 512], FP32, tag="pg", name="pg")
                        for ci in range(4):
                            c = cg * 4 + ci
                            o = psg[:, ci * 128:(ci + 1) * 128]
                            sl = slice(c * 128, (c + 1) * 128)
                            if t > 0:
                                nc.tensor.matmul(o, hT[:, sl], wgh[blk][:], start=True, stop=False)
                                nc.tensor.matmul(o, s12[:, sl], wgh12[blk][:], start=False, stop=False)
                            else:
                                nc.tensor.matmul(o, hT[DH:DH + 1, sl], wgh[blk][DH:DH + 1, :], start=True, stop=False)
                            nc.tensor.matmul(o, xt_t[:, sl], wgx0[blk][:], start=False, stop=False)
                            nc.tensor.matmul(o, p12_t[:, sl], wgx12[blk][:], start=False, stop=True)
                        nc.scalar.activation(g[:, cg * 512:(cg + 1) * 512], psg[:], AF.Sigmoid)

                    # rh = r * h (own)
                    if t > 0:
                        rh = scr.tile([128, NCO * DH], FP32, tag="rh", name="rh")
                        r_view = g[:].rearrange("p (c f) -> p c f", c=NCO)[:, :, 0:DH]
                        h_view = h[:].rearrange("p (c f) -> p c f", c=NCO)
                        rh_view = rh[:].rearrange("p (c f) -> p c f", c=NCO)
                        nc.vector.tensor_mul(rh_view, r_view, h_view)

                    # --- c path ---
                    if t > 0:
                        rhbf = scr.tile([128, NCO * DH], BF16, tag="rhbf", name="rhbf")
                        nc.vector.tensor_copy(rhbf[:], rh[:])
                        rhg = (RHG_a if t % 2 else RHG_b)[:]
                        gather(rhbf, rhg, AGIN_rh)
                        transpose_to(rhT, rhbf, DH)
                        rhf = to_sbuf_full(rhg)
                        s12c = amult(rhf, "s12c")

                    cc = scr.tile([128, NCO * DH], FP32, tag="cc", name="cc")
                    psc = pg.tile([128, 512], FP32, tag="pg", name="pgc")
                    for ci in range(8):
                        o = psc[:, ci * DH:(ci + 1) * DH]
                        sl = slice(ci * 128, (ci + 1) * 128)
                        if t > 0:
                            nc.tensor.matmul(o, rhT[:, sl], wch[blk][:], start=True, stop=False)
                            nc.tensor.matmul(o, s12c[:, sl], wch12[blk][:], start=False, stop=False)
                        else:
                            nc.tensor.matmul(o, rhT[DH:DH + 1, sl], wch[blk][DH:DH + 1, :], start=True, stop=False)
                        nc.tensor.matmul(o, xt_t[:, sl], wcx0[blk][:], start=False, stop=False)
                        nc.tensor.matmul(o, p12_t[:, sl], wcx12[blk][:], start=False, stop=True)
                    nc.scalar.activation(cc[:], psc[:], AF.Tanh)

                    # h_new = cc + u * (h - cc)
                    u_view = g[:].rearrange("p (c f) -> p c f", c=NCO)[:, :, DH:G]
                    hmc = scr.tile([128, NCO * DH], FP32, tag="rh", name="hmc")
                    nc.vector.tensor_sub(hmc[:], h[:], cc[:])
                    h_new = state.tile([128, NCO * DH], FP32, tag="h", name="hn")
                    hmc_view = hmc[:].rearrange("p (c f) -> p c f", c=NCO)
                    nc.vector.tensor_mul(hmc_view, u_view, hmc_view)
                    nc.vector.tensor_add(h_new[:], cc[:], hmc[:])
                    h = h_new

                if store_h1:
                    # final h: gather for H1G + transpose for H1T
                    hbf = scr.tile([128, NCO * DH], BF16, tag="hbfg", name="hbff")
                    nc.vector.tensor_copy(hbf[:], h[:])
                    gather(hbf, H1G_dr[(T - 1) * N:T * N, :], AGIN_h)
                    transpose_to(hT, hbf, DH)
                    nc.sync.dma_start(H1T_dr[(T - 1) * DH:T * DH, :], hT[0:DH, :])
                return h

            for _rep in range(repeat):
                precompute(0, load_lhsT_xf)
                dump_p(0)
                gru_block(0, XT_d, P1T_dr, P2T_dr, store_h1=True)
                precompute(1, load_lhsT_h1g)
                dump_p(1)
                h_fin = gru_block(1, H1T_dr, P1T_dr, P2T_dr, store_h1=False)

            nc.sync.dma_start(HOUT_d[:], h_fin[:])

    nc.finalize()
    return nc


# ---------------------------------------------------------------------------
# host-side preparation and execution
# ---------------------------------------------------------------------------

def _prep_inputs(X, A_x, Wg, bg, Wc, bc):
    f32 = np.float32
    A = A_x.astype(np.float64)
    A2 = A @ A
    AT = np.ascontiguousarray(A.T.astype(ml_dtypes.bfloat16))
    A2T = np.ascontiguousarray(A2.T.astype(ml_dtypes.bfloat16))

    def spec_norm(W):
        M = W.reshape(-1, W.shape[-1]).astype(np.float64)
        sigma = np.linalg.norm(M, ord=2)
        return (W.astype(np.float64) / sigma).astype(f32)

    WGH = np.zeros((NBLK, DH + 1, G), f32)
    WGH12 = np.zeros((NBLK, 2 * DH, G), f32)
    WGX0 = np.zeros((NBLK, DIN, G), f32)
    WGX12 = np.zeros((NBLK, 2 * DIN, G), f32)
    WCH = np.zeros((NBLK, DH + 1, DH), f32)
    WCH12 = np.zeros((NBLK, 2 * DH, DH), f32)
    WCX0 = np.zeros((NBLK, DIN, DH), f32)
    WCX12 = np.zeros((NBLK, 2 * DIN, DH), f32)
    for blk in range(NBLK):
        Wg_n = spec_norm(Wg[blk])
        Wc_n = spec_norm(Wc[blk])
        WGX0[blk] = Wg_n[0][:DIN]
        WGH[blk, :DH] = Wg_n[0][DIN:]
        WGH[blk, DH] = bg[blk]
        WGX12[blk, :DIN] = Wg_n[1][:DIN]
        WGX12[blk, DIN:] = Wg_n[2][:DIN]
        WGH12[blk, :DH] = Wg_n[1][DIN:]
        WGH12[blk, DH:] = Wg_n[2][DIN:]
        WCX0[blk] = Wc_n[0][:DIN]
        WCH[blk, :DH] = Wc_n[0][DIN:]
        WCH[blk, DH] = bc[blk]
        WCX12[blk, :DIN] = Wc_n[1][:DIN]
        WCX12[blk, DIN:] = Wc_n[2][:DIN]
        WCH12[blk, :DH] = Wc_n[1][DIN:]
        WCH12[blk, DH:] = Wc_n[2][DIN:]

    shared = {
        "WGH": WGH.astype(ml_dtypes.bfloat16), "WGH12": WGH12.astype(ml_dtypes.bfloat16),
        "WGX0": WGX0.astype(ml_dtypes.bfloat16), "WGX12": WGX12.astype(ml_dtypes.bfloat16),
        "WCH": WCH.astype(ml_dtypes.bfloat16), "WCH12": WCH12.astype(ml_dtypes.bfloat16),
        "WCX0": WCX0.astype(ml_dtypes.bfloat16), "WCX12": WCX12.astype(ml_dtypes.bfloat16),
    }

    in_maps = []
    for core in range(8):
        b = core % B
        half = core // B
        own = slice(half * NO, (half + 1) * NO)
        Xb = X[b]                                    # [T, N, DIN]
        XF = np.ascontiguousarray(
            Xb.reshape(T, NC, 128, DIN).transpose(2, 0, 1, 3).reshape(128, T * NC * DIN)
        ).astype(ml_dtypes.bfloat16)
        XT = np.ascontiguousarray(
            Xb.transpose(0, 2, 1).reshape(T * DIN, N)[:, own]).astype(ml_dtypes.bfloat16)
        im = dict(shared)
        im["AT"] = np.ascontiguousarray(AT[:, own])
        im["A2T"] = np.ascontiguousarray(A2T[:, own])
        im["XF"] = XF
        im["XT"] = XT
        in_maps.append(im)
    return in_maps


_CACHED = {}


def _get_nc(repeat=1, debug=False, dbg_blk=0, dbg_t=1):
    key = (repeat, debug, dbg_blk, dbg_t)
    if key not in _CACHED:
        _CACHED[key] = build_kernel(repeat, debug, dbg_blk, dbg_t)
    return _CACHED[key]


def run_on_device(inputs, repeat=1, time_iters=0, debug=False, dbg_blk=0, dbg_t=1, raw=False,
                  use_spmd_api=False):
    """Returns (per-batch final h [B, N, DH] fp32, wall_ns or None)."""
    nc = _get_nc(repeat, debug, dbg_blk, dbg_t)
    in_maps = _prep_inputs(inputs["X"], inputs["A_x"], inputs["Wg"], inputs["bg"],
                           inputs["Wc"], inputs["bc"])
    if use_spmd_api:
        from concourse import bass_utils as _bu
        res = _bu.run_bass_kernel_spmd(nc, in_maps, core_ids=list(range(8)), trace=False)
        results, wall = res.results, None
    else:
        from runner_embedded import make_runner
        run = make_runner(nc, 8)
        results, wall = run(in_maps, time_iters=time_iters)
    if raw:
        return results, wall
    hs = []
    for b in range(B):
        lo = results[b]["HOUT"].reshape(128, NCO, DH).transpose(1, 0, 2).reshape(NO, DH)
        hi = results[b + 4]["HOUT"].reshape(128, NCO, DH).transpose(1, 0, 2).reshape(NO, DH)
        hs.append(np.concatenate([lo, hi], axis=0))
    return np.stack(hs), wall


def kernel(**inputs):
    X = inputs["X"]
    W_out = inputs["W_out"].astype(np.float64)
    b_out = inputs["b_out"].astype(np.float64)
    hs, _ = run_on_device(inputs, use_spmd_api=True)
    W_sn = W_out / np.linalg.norm(W_out)
    pred = hs.astype(np.float64) @ W_sn + b_out     # [B, N, 1]
    out = pred.squeeze(-1).mean()
    return np.float32(out)


# ---- embedded runner (kernel.py must be self-contained) ----
import sys as _sys
import types as _types

_runner_src = '''
import time
import numpy as np
import jax
from jax.sharding import Mesh, PartitionSpec
from jax.experimental.shard_map import shard_map

import concourse.mybir as mybir
from concourse.bass2jax import _bass_exec_p, partition_id_tensor, install_neuronx_cc_hook


def make_runner(nc, n_cores):
    install_neuronx_cc_hook()
    partition_name = nc.partition_id_tensor.name if nc.partition_id_tensor else None

    in_names = []
    out_names = []
    out_avals = []
    zero_outs = []
    for alloc in nc.m.functions[0].allocations:
        if not isinstance(alloc, mybir.MemoryLocationSet):
            continue
        name = alloc.memorylocations[0].name
        if alloc.kind == "ExternalInput":
            if name != partition_name:
                in_names.append(name)
        elif alloc.kind == "ExternalOutput":
            out_names.append(name)
            shape = tuple(alloc.tensor_shape)
            dtype = mybir.dt.np(alloc.dtype)
            out_avals.append(jax.core.ShapedArray(shape, dtype))
            zero_outs.append(np.zeros(shape, dtype))
    n_params = len(in_names)
    n_outs = len(out_avals)
    all_in_names = list(in_names) + list(out_names)
    if partition_name is not None:
        all_in_names.append(partition_name)

    def _body(*args):
        operands = list(args)
        if partition_name is not None:
            operands.append(partition_id_tensor())
        outs = _bass_exec_p.bind(
            *operands,
            out_avals=tuple(out_avals),
            in_names=tuple(all_in_names),
            out_names=tuple(out_names),
            lowering_input_output_aliases=(),
            sim_require_finite=False,
            sim_require_nnan=False,
            nc=nc,
        )
        return tuple(outs)

    devices = jax.devices()[:n_cores]
    mesh = Mesh(np.asarray(devices), ("core",))
    in_specs = (PartitionSpec("core"),) * (n_params + n_outs)
    out_specs = (PartitionSpec("core"),) * len(out_names)
    sharded = jax.jit(
        shard_map(_body, mesh=mesh, in_specs=in_specs, out_specs=out_specs,
                  check_rep=False),
        keep_unused=True,
    )

    def run(in_maps, time_iters=0):
        per_core = [[np.asarray(m[name]) for name in in_names] for m in in_maps]
        concat_in = [
            np.concatenate([per_core[c][i] for c in range(n_cores)], axis=0)
            for i in range(n_params)
        ]
        concat_zeros = [
            np.zeros((n_cores * z.shape[0], *z.shape[1:]), z.dtype) for z in zero_outs
        ]
        out_arrs = sharded(*concat_in, *concat_zeros)
        jax.block_until_ready(out_arrs)
        wall_ns = None
        if time_iters:
            times = []
            for _ in range(time_iters):
                t0 = time.perf_counter_ns()
                out_arrs = sharded(*concat_in, *concat_zeros)
                jax.block_until_ready(out_arrs)
                times.append(time.perf_counter_ns() - t0)
            wall_ns = min(times)
        results = [
            {name: np.asarray(out_arrs[i]).reshape(n_cores, *out_avals[i].shape)[c]
             for i, name in enumerate(out_names)}
            for c in range(n_cores)
        ]
        return results, wall_ns

    return run
'''

_mod = _types.ModuleType("runner_embedded")
exec(_runner_src, _mod.__dict__)
_sys.modules["runner_embedded"] = _mod


if __name__ == "__main__":
    pass



# revision 2
# speedup vs baseline: 5.3082x; 5.3082x over previous
"""Trainium2 Bass kernel for nn_Discriminator (DCRNN-style GRU discriminator), v2.

Design (replaces the node-sharded v1 which serialized 29 x 15us AllGathers):
  - 8 cores = 4 batch-pairs. Core b runs GRU block 1 for batch b over the FULL
    graph; core b+4 runs block 2, software-pipelined SKEW slots behind.
  - Per slot, one small fp8 AllGather per pair hands h1(t) from the block-1
    core to the block-2 core; with SKEW=2 it has a full slot of slack and
    stays off the critical path. SPMD uniformity is kept by masking: both
    roles run identical instructions, A-cores multiply the received payload
    by 0, B-cores read their (host-zeroed) X as 0.
  - All A/A^2 matmuls run fp8-e4m3 with DoubleRow perf mode (2 K-tiles per
    pass). A and A^2 are pre-scaled by 2^17 on host (entries ~1e-3 would be
    subnormal in e4m3); the scale is divided out of the hop-1/2 gate weights.
  - dconv(xh) splits into x-terms and h-terms; the g-path A-matmul packs
    [xin | h] as a 128-wide stationary operand so both sides share one
    stream of A columns.
  - Gate matmuls and transposes in bf16, state/elementwise in fp32.
    Host does the final tiny pred = H[:,-1] @ W_sn + b_out and the mean.
"""
import numpy as np
import ml_dtypes

import concourse.bass as bass
import concourse.mybir as mybir
import concourse.tile as tile
from concourse import bacc
from concourse import bass_utils
from concourse.masks import make_identity

FP32 = mybir.dt.float32
BF16 = mybir.dt.bfloat16
F8 = mybir.dt.float8e4
AF = mybir.ActivationFunctionType
DR = mybir.MatmulPerfMode.DoubleRow

B, T, N, DIN, DH, K, NBLK = 4, 8, 2048, 64, 64, 3, 2
NC = N // 128     # 16 node chunks
G = 2 * DH        # 128 gate width
SKEW = 2
SLOTS = T + SKEW
SCALE = float(2 ** 17)


def build_kernel(trace_sim=False, dbg_slot=None):
    nc = bacc.Bacc(None, target_bir_lowering=False)

    AT8_d = nc.dram_tensor("AT8", [128, NC, N], F8, kind="ExternalInput")
    A2T8_d = nc.dram_tensor("A2T8", [128, NC, N], F8, kind="ExternalInput")
    XF_d = nc.dram_tensor("XF", [128, T, NC, DIN], BF16, kind="ExternalInput")
    RM_d = nc.dram_tensor("RM", [128, 1], FP32, kind="ExternalInput")
    ZM_d = nc.dram_tensor("ZM", [128, NC * DH], FP32, kind="ExternalInput")
    WG0X_d = nc.dram_tensor("WG0X", [DIN + 1, G], BF16, kind="ExternalInput")
    WG0H_d = nc.dram_tensor("WG0H", [DH, G], BF16, kind="ExternalInput")
    WG12_d = nc.dram_tensor("WG12", [128, 2, G], F8, kind="ExternalInput")
    WC0X_d = nc.dram_tensor("WC0X", [DIN + 1, DH], BF16, kind="ExternalInput")
    WC0H_d = nc.dram_tensor("WC0H", [DH, DH], BF16, kind="ExternalInput")
    WC12X_d = nc.dram_tensor("WC12X", [DIN, 2, DH], F8, kind="ExternalInput")
    WC12H_d = nc.dram_tensor("WC12H", [DH, 2, DH], F8, kind="ExternalInput")
    HOUT_d = nc.dram_tensor("HOUT", [128, NC * DH], FP32, kind="ExternalOutput")

    RG = [[b, b + 4] for b in range(B)]

    with tile.TileContext(nc, trace_sim=trace_sim) as tc:
        with (
            tc.tile_pool(name="big", bufs=1) as big,
            tc.tile_pool(name="st", bufs=2) as st,
            tc.tile_pool(name="wk", bufs=2) as wk,
            tc.tile_pool(name="w1", bufs=1) as w1,
            tc.tile_pool(name="psA", bufs=3, space="PSUM") as psA,
            tc.tile_pool(name="psG", bufs=2, space="PSUM") as psG,
            tc.tile_pool(name="psT", bufs=2, space="PSUM") as psT,
            tc.tile_pool(name="dram", bufs=1, space="DRAM") as dram,
        ):
            AT8 = big.tile([128, NC, N], F8)
            A2T8 = big.tile([128, NC, N], F8)
            XF = big.tile([128, T, NC, DIN], BF16)
            RM = big.tile([128, 1], FP32)
            ZM = big.tile([128, NC * DH], FP32)
            nc.sync.dma_start(XF[:, 0:1], XF_d[:, 0:1])
            for srcd, dstt in ((AT8_d, AT8), (A2T8_d, A2T8)):
                nc.sync.dma_start(dstt[:, :, 0:512], srcd[:, :, 0:512])

            ident32 = big.tile([128, 128], FP32)
            make_identity(nc, ident32[:])
            ident = big.tile([128, 128], BF16)
            nc.vector.tensor_copy(ident[:], ident32[:])
            ones_bf = big.tile([1, N], BF16)
            nc.gpsimd.memset(ones_bf[:], 1.0)

            def wt(dram_t, p, f, nm):
                tl = big.tile([p, f], BF16, name=nm, tag=nm)
                nc.sync.dma_start(tl[:], dram_t[:])
                return tl
            WG0X = wt(WG0X_d, DIN + 1, G, "wg0x")
            WG0H = wt(WG0H_d, DH, G, "wg0h")
            WC0X = wt(WC0X_d, DIN + 1, DH, "wc0x")
            WC0H = wt(WC0H_d, DH, DH, "wc0h")
            WG12 = big.tile([128, 2, G], F8, name="wg12", tag="wg12")
            nc.sync.dma_start(WG12[:], WG12_d[:])
            WC12X = big.tile([DIN, 2, DH], F8, name="wc12x", tag="wc12x")
            nc.sync.dma_start(WC12X[:], WC12X_d[:])
            WC12H = big.tile([DH, 2, DH], F8, name="wc12h", tag="wc12h")
            nc.sync.dma_start(WC12H[:], WC12H_d[:])

            # rest of XF, masks, then remaining A stripes in consumption order
            nc.sync.dma_start(XF[:, 1:T], XF_d[:, 1:T])
            nc.sync.dma_start(RM[:], RM_d[:])
            nc.sync.dma_start(ZM[:], ZM_d[:])
            for j in range(1, 4):
                for src, dst in ((AT8_d, AT8), (A2T8_d, A2T8)):
                    nc.sync.dma_start(
                        dst[:, :, j * 512:(j + 1) * 512],
                        src[:, :, j * 512:(j + 1) * 512])

            AGIN = [dram.tile([N, DH], F8, name=f"agin{i}") for i in range(3)]
            AGOUT = [dram.tile([2 * N, DH], F8, name=f"agout{i}") for i in range(3)]

            h = st.tile([128, NC * DH], FP32, tag="h", name="h_init")
            nc.gpsimd.memset(h[:], 0.0)

            def transpose_fm(src_nm, dst, sfx, dt=BF16):
                # src_nm [128, NC*64] node-major -> dst rows 0:64 feat-major
                # [64, N]; transpose two chunks per pass ([128,128] blocks:
                # rows 0:64 = even chunk's feats, 64:128 = odd chunk's)
                for q in range(2):
                    pt = psT.tile([128, 4 * 128], dt, tag="pt", name="pt" + sfx)
                    for ci in range(4):
                        c2 = q * 4 + ci
                        nc.tensor.transpose(
                            pt[:, ci * 128:(ci + 1) * 128],
                            src_nm[:, c2 * 2 * DH:(c2 + 1) * 2 * DH],
                            ident32[:] if dt is FP32 else ident[:])
                    dv = dst[0:DH, q * 1024:(q + 1) * 1024].rearrange(
                        "f (k two p) -> f k two p", two=2, p=128)
                    ptv = pt[:].rearrange("r (k p) -> r k p", p=128)
                    if q == 0:
                        nc.vector.tensor_copy(dv[:, :, 0], ptv[0:DH])
                        nc.scalar.activation(dv[:, :, 1], ptv[DH:128], AF.Copy)
                    else:
                        nc.scalar.activation(dv[:, :, 0], ptv[0:DH], AF.Copy)
                        nc.vector.tensor_copy(dv[:, :, 1], ptv[DH:128])

            for s in range(SLOTS):
                t = s % T
                sfx = f"_s{s}"

                # --- receive partner payload first (so the SP queue serves it
                # before this slot's 15us+ collective occupies the queue)
                xin = wk.tile([128, NC * DIN], BF16, tag="xin", name="xin" + sfx)
                xin_nm = xin[:].rearrange("p (c f) -> p c f", c=NC)
                if s >= SKEW:
                    recv8 = wk.tile([128, NC, DIN], F8, tag="rcv", name="rcv" + sfx)
                    nc.sync.dma_start(
                        recv8[:],
                        AGOUT[(s - 1) % 3][0:N, :].rearrange("(c p) f -> p c f", c=NC))
                    # xin = role * recv + XF[t]   (role: 0 on block-1 cores)
                    for hq in range(2):
                        cs_ = slice(hq * 8, (hq + 1) * 8)
                        nc.vector.scalar_tensor_tensor(
                            xin_nm[:, cs_], recv8[:, cs_], RM[:], XF[:, t, cs_],
                            op0=mybir.AluOpType.mult, op1=mybir.AluOpType.add)
                else:
                    nc.vector.tensor_copy(xin_nm, XF[:, t])

                if s == SKEW:
                    hm = wk.tile([128, NC * DH], FP32, tag="h", name="hm")
                    nc.vector.tensor_mul(hm[:], h[:], ZM[:])
                    h = hm

                h_nm = h[:].rearrange("p (c f) -> p c f", c=NC)
                LH = wk.tile([128, NC, 128], F8, tag="LH", name="LH" + sfx)
                nc.vector.tensor_copy(LH[:, 0:8, DIN:128], h_nm[:, 0:8])
                nc.vector.tensor_copy(LH[:, 8:NC, DIN:128], h_nm[:, 8:NC])

                if 1 <= s <= T:
                    agv = AGIN[s % 3][:].rearrange("(c p) f -> p c f", c=NC)
                    nc.sync.dma_start(agv[:, 0:8], LH[:, 0:8, DIN:128])
                    nc.sync.dma_start(agv[:, 8:NC], LH[:, 8:NC, DIN:128])
                    nc.gpsimd.collective_compute(
                        "AllGather", mybir.AluOpType.bypass,
                        ins=[AGIN[s % 3][:]], outs=[AGOUT[s % 3][:]],
                        replica_groups=RG)

                nc.vector.tensor_copy(LH[:, 0:8, 0:DIN], xin_nm[:, 0:8])
                nc.vector.tensor_copy(LH[:, 8:NC, 0:DIN], xin_nm[:, 8:NC])
                hbf = wk.tile([128, NC * DH], BF16, tag="hbf", name="hbf" + sfx)
                nc.scalar.activation(hbf[:], h[:], AF.Copy)

                XIN = wk.tile([DIN + 1, N], BF16, tag="XIN", name="XIN" + sfx)
                transpose_fm(xin, XIN, "x" + sfx)
                nc.vector.tensor_copy(XIN[DIN:DIN + 1, :], ones_bf[:])
                HT = wk.tile([DH, N], BF16, tag="HT", name="HT" + sfx)
                transpose_fm(hbf, HT, "h" + sfx)

                # --- packed [xin | h] diffusion + g gates, interleaved per
                # 512-col j-block so gates/sigmoid/rh trail the A stream.
                # Hop-1/2 results live as DoubleRow K-pairs in one fp8 tile,
                # scaled 2^-7 (psum carries 2^17 from A, stored s*2^10).
                P12 = w1.tile([128, 2, N], F8, tag="P12", name="P12" + sfx)
                g = w1.tile([128, NC * G], FP32, tag="g", name="g" + sfx)
                g_nm = g[:].rearrange("p (c f) -> p c f", c=NC)
                rh = wk.tile([128, NC * DH], FP32, tag="rh", name="rh" + sfx)
                rh_nm = rh[:].rearrange("p (c f) -> p c f", c=NC)
                rh8 = wk.tile([128, NC, DH], F8, tag="rh8", name="rh8" + sfx)
                for j in range(4):
                    js = slice(j * 512, (j + 1) * 512)
                    for hop, Asrc in enumerate((AT8, A2T8)):
                        pa = psA.tile([128, 512], FP32, tag="pa", name="pa" + sfx)
                        for kk in range(8):
                            nc.tensor.matmul(
                                pa[:],
                                LH[:, 2 * kk:2 * kk + 2, :],
                                Asrc[:, 2 * kk:2 * kk + 2, js],
                                start=(kk == 0), stop=(kk == 7),
                                perf_mode=DR)
                        if hop == 0:
                            nc.vector.tensor_scalar_mul(
                                P12[:, 0, js], pa[:], 2.0 ** -7)
                        else:
                            nc.scalar.activation(
                                P12[:, 1, js], pa[:], AF.Copy, scale=2.0 ** -7)
                    psg = psG.tile([128, 512], FP32, tag="psg", name="psg" + sfx)
                    for ci in range(4):
                        c = j * 4 + ci
                        o = psg[:, ci * G:(ci + 1) * G]
                        sl = slice(c * 128, (c + 1) * 128)
                        nc.tensor.matmul(o, XIN[:, sl], WG0X[:], start=True, stop=False)
                        nc.tensor.matmul(o, HT[:, sl], WG0H[:], start=False, stop=False)
                        nc.tensor.matmul(o, P12[:, :, sl], WG12[:], start=False,
                                         stop=True, perf_mode=DR)
                    nc.scalar.activation(g[:, js], psg[:], AF.Sigmoid,
                                         scale=2.0 ** -10)
                    cs = slice(j * 4, (j + 1) * 4)
                    nc.vector.tensor_mul(
                        rh_nm[:, cs], g_nm[:, cs, 0:DH], h_nm[:, cs])
                    nc.vector.tensor_copy(rh8[:, cs], rh_nm[:, cs])
                rhbf = wk.tile([128, NC * DH], BF16, tag="rhbf", name="rhbf" + sfx)
                nc.scalar.activation(rhbf[:], rh[:], AF.Copy)
                RHT = wk.tile([DH, N], BF16, tag="RHT", name="RHT" + sfx)
                transpose_fm(rhbf, RHT, "r" + sfx)

                # --- c diffusion + c gates + h update, interleaved: 2 j-blocks
                # of SR per gate bank, then tanh + update per half
                SR12 = w1.tile([DH, 2, N], F8, tag="SR12", name="SR12" + sfx)
                cc = w1.tile([128, NC * DH], FP32, tag="cc", name="cc" + sfx)
                hmc = wk.tile([128, NC * DH], FP32, tag="hmc", name="hmc" + sfx)
                hmc_nm = hmc[:].rearrange("p (c f) -> p c f", c=NC)
                h_new = st.tile([128, NC * DH], FP32, tag="h", name="hn" + sfx)
                for q in range(2):
                    for jj in range(2):
                        j = q * 2 + jj
                        js = slice(j * 512, (j + 1) * 512)
                        for hop, Asrc in enumerate((AT8, A2T8)):
                            pc = psA.tile([DH, 512], FP32, tag="pa", name="pc" + sfx)
                            for kk in range(8):
                                nc.tensor.matmul(
                                    pc[:],
                                    rh8[:, 2 * kk:2 * kk + 2, :],
                                    Asrc[:, 2 * kk:2 * kk + 2, js],
                                    start=(kk == 0), stop=(kk == 7),
                                    perf_mode=DR)
                            if hop == 0:
                                nc.vector.tensor_scalar_mul(
                                    SR12[:, 0, js], pc[:], 2.0 ** -7)
                            else:
                                nc.scalar.activation(
                                    SR12[:, 1, js], pc[:], AF.Copy,
                                    scale=2.0 ** -7)
                    psc = psG.tile([128, 512], FP32, tag="psg", name="psc" + sfx)
                    for ci in range(8):
                        c = q * 8 + ci
                        o = psc[:, ci * DH:(ci + 1) * DH]
                        sl = slice(c * 128, (c + 1) * 128)
                        nc.tensor.matmul(o, XIN[:, sl], WC0X[:], start=True, stop=False)
                        nc.tensor.matmul(o, RHT[:, sl], WC0H[:], start=False, stop=False)
                        nc.tensor.matmul(o, P12[0:DIN, :, sl], WC12X[:],
                                         start=False, stop=False, perf_mode=DR)
                        nc.tensor.matmul(o, SR12[:, :, sl], WC12H[:],
                                         start=False, stop=True, perf_mode=DR)
                    hs_ = slice(q * 512, (q + 1) * 512)
                    cs_ = slice(q * 8, (q + 1) * 8)
                    nc.scalar.activation(cc[:, hs_], psc[:], AF.Tanh,
                                         scale=2.0 ** -10)
                    eng = nc.vector if q == 0 else nc.gpsimd
                    eng.tensor_sub(hmc[:, hs_], h[:, hs_], cc[:, hs_])
                    eng.tensor_mul(
                        hmc_nm[:, cs_], g_nm[:, cs_, DH:G], hmc_nm[:, cs_])
                    eng.tensor_add(h_new[:, hs_], cc[:, hs_], hmc[:, hs_])
                h = h_new

                if dbg_slot is not None and s == dbg_slot:
                    break

            nc.sync.dma_start(HOUT_d[:], h[:])

    nc.finalize()
    return nc


# ---------------------------------------------------------------------------
# host-side preparation and execution
# ---------------------------------------------------------------------------

def _prep_inputs(X, A_x, Wg, bg, Wc, bc):
    f32 = np.float32
    bf = ml_dtypes.bfloat16
    f8 = ml_dtypes.float8_e4m3
    A = A_x.astype(np.float64)
    A2 = A @ A

    # rhs layout [128, NC, N]: arr[p, k, n] = A[n, k*128+p] * SCALE
    def a_rhs(M):
        return np.ascontiguousarray(
            (M.T * SCALE).reshape(NC, 128, N).transpose(1, 0, 2)).astype(f8)
    AT8 = a_rhs(A)
    A2T8 = a_rhs(A2)

    def spec_norm(W):
        M = W.reshape(-1, W.shape[-1]).astype(np.float64)
        return W.astype(np.float64) / np.linalg.norm(M, ord=2)

    blk_w = []
    for blk in range(NBLK):
        Wg_n = spec_norm(Wg[blk])
        Wc_n = spec_norm(Wc[blk])
        # bf16 0-hop weights carry 2^10 (gate psum scale; activations apply
        # 2^-10). fp8 hop-1/2 weights are unscaled: inputs arrive as s*2^10.
        PS = 2.0 ** 10
        WG0X = np.zeros((DIN + 1, G), f32)
        WG0X[:DIN] = Wg_n[0][:DIN] * PS
        WG0X[DIN] = bg[blk] * PS
        WG0H = (Wg_n[0][DIN:] * PS).astype(f32)
        WG12 = np.stack([
            np.concatenate([Wg_n[1][:DIN], Wg_n[1][DIN:]], 0),
            np.concatenate([Wg_n[2][:DIN], Wg_n[2][DIN:]], 0)], 1)
        WC0X = np.zeros((DIN + 1, DH), f32)
        WC0X[:DIN] = Wc_n[0][:DIN] * PS
        WC0X[DIN] = bc[blk] * PS
        WC0H = (Wc_n[0][DIN:] * PS).astype(f32)
        WC12X = np.stack([Wc_n[1][:DIN], Wc_n[2][:DIN]], 1)
        WC12H = np.stack([Wc_n[1][DIN:], Wc_n[2][DIN:]], 1)
        blk_w.append({
            "WG0X": WG0X.astype(bf), "WG0H": WG0H.astype(bf),
            "WG12": WG12.astype(f8),
            "WC0X": WC0X.astype(bf), "WC0H": WC0H.astype(bf),
            "WC12X": WC12X.astype(f8), "WC12H": WC12H.astype(f8),
        })

    zeros_xf = np.zeros((128, T, NC, DIN), bf)
    in_maps = []
    for core in range(8):
        b = core % B
        role = core // B  # 0 = block-1 runner, 1 = block-2 runner
        im = dict(blk_w[role])
        im["AT8"] = AT8
        im["A2T8"] = A2T8
        if role == 0:
            im["XF"] = np.ascontiguousarray(
                X[b].reshape(T, NC, 128, DIN).transpose(2, 0, 1, 3)).astype(bf)
        else:
            im["XF"] = zeros_xf
        im["RM"] = np.full((128, 1), float(role), f32)
        im["ZM"] = np.full((128, NC * DH), float(1 - role), f32)
        in_maps.append(im)
    return in_maps


_CACHED = {}


def _get_nc(trace_sim=False, dbg_slot=None):
    key = (trace_sim, dbg_slot)
    if key not in _CACHED:
        _CACHED[key] = build_kernel(trace_sim, dbg_slot)
    return _CACHED[key]


def run_on_device(inputs, dbg_slot=None):
    """Returns per-batch final h [B, N, DH] fp32 (block-2 cores' HOUT)."""
    nc = _get_nc(dbg_slot=dbg_slot)
    in_maps = _prep_inputs(inputs["X"], inputs["A_x"], inputs["Wg"], inputs["bg"],
                           inputs["Wc"], inputs["bc"])
    res = bass_utils.run_bass_kernel_spmd(nc, in_maps, core_ids=list(range(8)))
    results = res.results
    hs = []
    for b in range(B):
        hb = results[4 + b]["HOUT"].reshape(128, NC, DH).transpose(1, 0, 2)
        hs.append(hb.reshape(N, DH))
    return np.stack(hs), results


def kernel(**inputs):
    W_out = inputs["W_out"].astype(np.float64)
    b_out = inputs["b_out"].astype(np.float64)
    hs, _ = run_on_device(inputs)
    W_sn = W_out / np.linalg.norm(W_out)
    pred = hs.astype(np.float64) @ W_sn + b_out     # [B, N, 1]
    out = pred.squeeze(-1).mean()
    return np.float32(out)


if __name__ == "__main__":
    pass


# revision 3
# speedup vs baseline: 5.3093x; 1.0002x over previous
"""Trainium2 Bass kernel for nn_Discriminator (DCRNN-style GRU discriminator), v2.

Design (replaces the node-sharded v1 which serialized 29 x 15us AllGathers):
  - 8 cores = 4 batch-pairs. Core b runs GRU block 1 for batch b over the FULL
    graph; core b+4 runs block 2, software-pipelined SKEW slots behind.
  - Per slot, one small fp8 AllGather per pair hands h1(t) from the block-1
    core to the block-2 core; with SKEW=2 it has a full slot of slack and
    stays off the critical path. SPMD uniformity is kept by masking: both
    roles run identical instructions, A-cores multiply the received payload
    by 0, B-cores read their (host-zeroed) X as 0.
  - All A/A^2 matmuls run fp8-e4m3 with DoubleRow perf mode (2 K-tiles per
    pass). A and A^2 are pre-scaled by 2^17 on host (entries ~1e-3 would be
    subnormal in e4m3); the scale is divided out of the hop-1/2 gate weights.
  - dconv(xh) splits into x-terms and h-terms; the g-path A-matmul packs
    [xin | h] as a 128-wide stationary operand so both sides share one
    stream of A columns.
  - Gate matmuls and transposes in bf16, state/elementwise in fp32.
    Host does the final tiny pred = H[:,-1] @ W_sn + b_out and the mean.
"""
import numpy as np
import ml_dtypes

import concourse.bass as bass
import concourse.mybir as mybir
import concourse.tile as tile
from concourse import bacc
from concourse import bass_utils
from concourse.masks import make_identity

FP32 = mybir.dt.float32
BF16 = mybir.dt.bfloat16
F8 = mybir.dt.float8e4
AF = mybir.ActivationFunctionType
DR = mybir.MatmulPerfMode.DoubleRow

B, T, N, DIN, DH, K, NBLK = 4, 8, 2048, 64, 64, 3, 2
NC = N // 128     # 16 node chunks
G = 2 * DH        # 128 gate width
SKEW = 2
SLOTS = T + SKEW
SCALE = float(2 ** 17)


def build_kernel(trace_sim=False, dbg_slot=None):
    nc = bacc.Bacc(None, target_bir_lowering=False)

    AT8_d = nc.dram_tensor("AT8", [128, NC, N], F8, kind="ExternalInput")
    A2T8_d = nc.dram_tensor("A2T8", [128, NC, N], F8, kind="ExternalInput")
    XF_d = nc.dram_tensor("XF", [128, T, NC, DIN], BF16, kind="ExternalInput")
    RM_d = nc.dram_tensor("RM", [128, 1], FP32, kind="ExternalInput")
    ZM_d = nc.dram_tensor("ZM", [128, NC * DH], FP32, kind="ExternalInput")
    WG0X_d = nc.dram_tensor("WG0X", [DIN + 1, G], BF16, kind="ExternalInput")
    WG0H_d = nc.dram_tensor("WG0H", [DH, G], BF16, kind="ExternalInput")
    WG12_d = nc.dram_tensor("WG12", [128, 2, G], F8, kind="ExternalInput")
    WC0X_d = nc.dram_tensor("WC0X", [DIN + 1, DH], BF16, kind="ExternalInput")
    WC0H_d = nc.dram_tensor("WC0H", [DH, DH], BF16, kind="ExternalInput")
    WC12X_d = nc.dram_tensor("WC12X", [DIN, 2, DH], F8, kind="ExternalInput")
    WC12H_d = nc.dram_tensor("WC12H", [DH, 2, DH], F8, kind="ExternalInput")
    HOUT_d = nc.dram_tensor("HOUT", [128, NC * DH], FP32, kind="ExternalOutput")

    RG = [[b, b + 4] for b in range(B)]

    with tile.TileContext(nc, trace_sim=trace_sim) as tc:
        with (
            tc.tile_pool(name="big", bufs=1) as big,
            tc.tile_pool(name="st", bufs=2) as st,
            tc.tile_pool(name="wk", bufs=2) as wk,
            tc.tile_pool(name="w1", bufs=1) as w1,
            tc.tile_pool(name="psA", bufs=3, space="PSUM") as psA,
            tc.tile_pool(name="psG", bufs=3, space="PSUM") as psG,
            tc.tile_pool(name="psT", bufs=2, space="PSUM") as psT,
            tc.tile_pool(name="dram", bufs=1, space="DRAM") as dram,
        ):
            AT8 = big.tile([128, NC, N], F8)
            A2T8 = big.tile([128, NC, N], F8)
            XF = big.tile([128, T, NC, DIN], BF16)
            RM = big.tile([128, 1], FP32)
            ZM = big.tile([128, NC * DH], FP32)
            nc.sync.dma_start(XF[:, 0:1], XF_d[:, 0:1])
            for srcd, dstt in ((AT8_d, AT8), (A2T8_d, A2T8)):
                nc.sync.dma_start(dstt[:, :, 0:512], srcd[:, :, 0:512])

            ident32 = big.tile([128, 128], FP32)
            make_identity(nc, ident32[:])
            ident = big.tile([128, 128], BF16)
            nc.vector.tensor_copy(ident[:], ident32[:])
            ones_bf = big.tile([1, N], BF16)
            nc.gpsimd.memset(ones_bf[:], 1.0)

            def wt(dram_t, p, f, nm):
                tl = big.tile([p, f], BF16, name=nm, tag=nm)
                nc.sync.dma_start(tl[:], dram_t[:])
                return tl
            WG0X = wt(WG0X_d, DIN + 1, G, "wg0x")
            WG0H = wt(WG0H_d, DH, G, "wg0h")
            WC0X = wt(WC0X_d, DIN + 1, DH, "wc0x")
            WC0H = wt(WC0H_d, DH, DH, "wc0h")
            WG12 = big.tile([128, 2, G], F8, name="wg12", tag="wg12")
            nc.sync.dma_start(WG12[:], WG12_d[:])
            WC12X = big.tile([DIN, 2, DH], F8, name="wc12x", tag="wc12x")
            nc.sync.dma_start(WC12X[:], WC12X_d[:])
            WC12H = big.tile([DH, 2, DH], F8, name="wc12h", tag="wc12h")
            nc.sync.dma_start(WC12H[:], WC12H_d[:])

            # rest of XF, masks, then remaining A stripes in consumption order
            nc.sync.dma_start(XF[:, 1:T], XF_d[:, 1:T])
            nc.sync.dma_start(RM[:], RM_d[:])
            nc.sync.dma_start(ZM[:], ZM_d[:])
            for j in range(1, 4):
                for src, dst in ((AT8_d, AT8), (A2T8_d, A2T8)):
                    nc.sync.dma_start(
                        dst[:, :, j * 512:(j + 1) * 512],
                        src[:, :, j * 512:(j + 1) * 512])

            AGIN = [dram.tile([N, DH], F8, name=f"agin{i}") for i in range(3)]
            AGOUT = [dram.tile([2 * N, DH], F8, name=f"agout{i}") for i in range(3)]

            h = st.tile([128, NC * DH], FP32, tag="h", name="h_init")
            nc.gpsimd.memset(h[:], 0.0)

            def transpose_fm(src_nm, dst, sfx, dt=BF16):
                # src_nm [128, NC*64] node-major -> dst rows 0:64 feat-major
                # [64, N]; transpose two chunks per pass ([128,128] blocks:
                # rows 0:64 = even chunk's feats, 64:128 = odd chunk's)
                for q in range(2):
                    pt = psT.tile([128, 4 * 128], dt, tag="pt", name="pt" + sfx)
                    for ci in range(4):
                        c2 = q * 4 + ci
                        nc.tensor.transpose(
                            pt[:, ci * 128:(ci + 1) * 128],
                            src_nm[:, c2 * 2 * DH:(c2 + 1) * 2 * DH],
                            ident32[:] if dt is FP32 else ident[:])
                    dv = dst[0:DH, q * 1024:(q + 1) * 1024].rearrange(
                        "f (k two p) -> f k two p", two=2, p=128)
                    ptv = pt[:].rearrange("r (k p) -> r k p", p=128)
                    if q == 0:
                        nc.vector.tensor_copy(dv[:, :, 0], ptv[0:DH])
                        nc.scalar.activation(dv[:, :, 1], ptv[DH:128], AF.Copy)
                    else:
                        nc.scalar.activation(dv[:, :, 0], ptv[0:DH], AF.Copy)
                        nc.vector.tensor_copy(dv[:, :, 1], ptv[DH:128])

            for s in range(SLOTS):
                t = s % T
                sfx = f"_s{s}"

                # --- receive partner payload first (so the SP queue serves it
                # before this slot's 15us+ collective occupies the queue)
                xin = wk.tile([128, NC * DIN], BF16, tag="xin", name="xin" + sfx)
                xin_nm = xin[:].rearrange("p (c f) -> p c f", c=NC)
                if s >= SKEW:
                    recv8 = wk.tile([128, NC, DIN], F8, tag="rcv", name="rcv" + sfx)
                    nc.sync.dma_start(
                        recv8[:],
                        AGOUT[(s - 1) % 3][0:N, :].rearrange("(c p) f -> p c f", c=NC))
                    # xin = role * recv + XF[t]   (role: 0 on block-1 cores)
                    for hq in range(2):
                        cs_ = slice(hq * 8, (hq + 1) * 8)
                        nc.vector.scalar_tensor_tensor(
                            xin_nm[:, cs_], recv8[:, cs_], RM[:], XF[:, t, cs_],
                            op0=mybir.AluOpType.mult, op1=mybir.AluOpType.add)
                else:
                    nc.vector.tensor_copy(xin_nm, XF[:, t])

                if s == SKEW:
                    hm = wk.tile([128, NC * DH], FP32, tag="h", name="hm")
                    nc.vector.tensor_mul(hm[:], h[:], ZM[:])
                    h = hm

                h_nm = h[:].rearrange("p (c f) -> p c f", c=NC)
                LH = wk.tile([128, NC, 128], F8, tag="LH", name="LH" + sfx)
                nc.vector.tensor_copy(LH[:, 0:8, DIN:128], h_nm[:, 0:8])
                nc.vector.tensor_copy(LH[:, 8:NC, DIN:128], h_nm[:, 8:NC])

                if 1 <= s <= T:
                    agv = AGIN[s % 3][:].rearrange("(c p) f -> p c f", c=NC)
                    nc.sync.dma_start(agv[:, 0:8], LH[:, 0:8, DIN:128])
                    nc.sync.dma_start(agv[:, 8:NC], LH[:, 8:NC, DIN:128])
                    nc.gpsimd.collective_compute(
                        "AllGather", mybir.AluOpType.bypass,
                        ins=[AGIN[s % 3][:]], outs=[AGOUT[s % 3][:]],
                        replica_groups=RG)

                nc.vector.tensor_copy(LH[:, 0:8, 0:DIN], xin_nm[:, 0:8])
                nc.vector.tensor_copy(LH[:, 8:NC, 0:DIN], xin_nm[:, 8:NC])
                hbf = wk.tile([128, NC * DH], BF16, tag="hbf", name="hbf" + sfx)
                nc.scalar.activation(hbf[:], h[:], AF.Copy)

                XIN = wk.tile([DIN + 1, N], BF16, tag="XIN", name="XIN" + sfx)
                transpose_fm(xin, XIN, "x" + sfx)
                nc.vector.tensor_copy(XIN[DIN:DIN + 1, :], ones_bf[:])
                HT = wk.tile([DH, N], BF16, tag="HT", name="HT" + sfx)
                transpose_fm(hbf, HT, "h" + sfx)

                # --- packed [xin | h] diffusion + g gates, interleaved per
                # 512-col j-block so gates/sigmoid/rh trail the A stream.
                # Hop-1/2 results live as DoubleRow K-pairs in one fp8 tile,
                # scaled 2^-7 (psum carries 2^17 from A, stored s*2^10).
                P12 = w1.tile([128, 2, N], F8, tag="P12", name="P12" + sfx)
                g = w1.tile([128, NC * G], FP32, tag="g", name="g" + sfx)
                g_nm = g[:].rearrange("p (c f) -> p c f", c=NC)
                rh = wk.tile([128, NC * DH], FP32, tag="rh", name="rh" + sfx)
                rh_nm = rh[:].rearrange("p (c f) -> p c f", c=NC)
                rh8 = wk.tile([128, NC, DH], F8, tag="rh8", name="rh8" + sfx)
                for j in range(4):
                    js = slice(j * 512, (j + 1) * 512)
                    for hop, Asrc in enumerate((AT8, A2T8)):
                        pa = psA.tile([128, 512], FP32, tag="pa", name="pa" + sfx)
                        for kk in range(8):
                            nc.tensor.matmul(
                                pa[:],
                                LH[:, 2 * kk:2 * kk + 2, :],
                                Asrc[:, 2 * kk:2 * kk + 2, js],
                                start=(kk == 0), stop=(kk == 7),
                                perf_mode=DR)
                        if hop == 0:
                            nc.vector.tensor_scalar_mul(
                                P12[:, 0, js], pa[:], 2.0 ** -7)
                        else:
                            nc.scalar.activation(
                                P12[:, 1, js], pa[:], AF.Copy, scale=2.0 ** -7)
                    psg = psG.tile([128, 512], FP32, tag="psg", name="psg" + sfx)
                    for ci in range(4):
                        c = j * 4 + ci
                        o = psg[:, ci * G:(ci + 1) * G]
                        sl = slice(c * 128, (c + 1) * 128)
                        nc.tensor.matmul(o, XIN[:, sl], WG0X[:], start=True, stop=False)
                        nc.tensor.matmul(o, HT[:, sl], WG0H[:], start=False, stop=False)
                        nc.tensor.matmul(o, P12[:, :, sl], WG12[:], start=False,
                                         stop=True, perf_mode=DR)
                    nc.scalar.activation(g[:, js], psg[:], AF.Sigmoid,
                                         scale=2.0 ** -10)
                    cs = slice(j * 4, (j + 1) * 4)
                    nc.vector.tensor_mul(
                        rh_nm[:, cs], g_nm[:, cs, 0:DH], h_nm[:, cs])
                    nc.vector.tensor_copy(rh8[:, cs], rh_nm[:, cs])
                rhbf = wk.tile([128, NC * DH], BF16, tag="rhbf", name="rhbf" + sfx)
                nc.scalar.activation(rhbf[:], rh[:], AF.Copy)
                RHT = wk.tile([DH, N], BF16, tag="RHT", name="RHT" + sfx)
                transpose_fm(rhbf, RHT, "r" + sfx)

                # --- c diffusion + c gates + h update, interleaved: 2 j-blocks
                # of SR per gate bank, then tanh + update per half
                SR12 = w1.tile([DH, 2, N], F8, tag="SR12", name="SR12" + sfx)
                cc = w1.tile([128, NC * DH], FP32, tag="cc", name="cc" + sfx)
                hmc = wk.tile([128, NC * DH], FP32, tag="hmc", name="hmc" + sfx)
                hmc_nm = hmc[:].rearrange("p (c f) -> p c f", c=NC)
                h_new = st.tile([128, NC * DH], FP32, tag="h", name="hn" + sfx)
                for q in range(2):
                    for jj in range(2):
                        j = q * 2 + jj
                        js = slice(j * 512, (j + 1) * 512)
                        for hop, Asrc in enumerate((AT8, A2T8)):
                            pc = psA.tile([DH, 512], FP32, tag="pa", name="pc" + sfx)
                            for kk in range(8):
                                nc.tensor.matmul(
                                    pc[:],
                                    rh8[:, 2 * kk:2 * kk + 2, :],
                                    Asrc[:, 2 * kk:2 * kk + 2, js],
                                    start=(kk == 0), stop=(kk == 7),
                                    perf_mode=DR)
                            if hop == 0:
                                nc.vector.tensor_scalar_mul(
                                    SR12[:, 0, js], pc[:], 2.0 ** -7)
                            else:
                                nc.scalar.activation(
                                    SR12[:, 1, js], pc[:], AF.Copy,
                                    scale=2.0 ** -7)
                    psc = psG.tile([128, 512], FP32, tag="psg", name="psc" + sfx)
                    for ci in range(8):
                        c = q * 8 + ci
                        o = psc[:, ci * DH:(ci + 1) * DH]
                        sl = slice(c * 128, (c + 1) * 128)
                        nc.tensor.matmul(o, XIN[:, sl], WC0X[:], start=True, stop=False)
                        nc.tensor.matmul(o, RHT[:, sl], WC0H[:], start=False, stop=False)
                        nc.tensor.matmul(o, P12[0:DIN, :, sl], WC12X[:],
                                         start=False, stop=False, perf_mode=DR)
                        nc.tensor.matmul(o, SR12[:, :, sl], WC12H[:],
                                         start=False, stop=True, perf_mode=DR)
                    hs_ = slice(q * 512, (q + 1) * 512)
                    cs_ = slice(q * 8, (q + 1) * 8)
                    nc.scalar.activation(cc[:, hs_], psc[:], AF.Tanh,
                                         scale=2.0 ** -10)
                    eng = nc.vector if q == 0 else nc.gpsimd
                    eng.tensor_sub(hmc[:, hs_], h[:, hs_], cc[:, hs_])
                    eng.tensor_mul(
                        hmc_nm[:, cs_], g_nm[:, cs_, DH:G], hmc_nm[:, cs_])
                    eng.tensor_add(h_new[:, hs_], cc[:, hs_], hmc[:, hs_])
                h = h_new

                if dbg_slot is not None and s == dbg_slot:
                    break

            nc.sync.dma_start(HOUT_d[:], h[:])

    nc.finalize()
    return nc


# ---------------------------------------------------------------------------
# host-side preparation and execution
# ---------------------------------------------------------------------------

def _prep_inputs(X, A_x, Wg, bg, Wc, bc):
    f32 = np.float32
    bf = ml_dtypes.bfloat16
    f8 = ml_dtypes.float8_e4m3
    A = A_x.astype(np.float64)
    A2 = A @ A

    # rhs layout [128, NC, N]: arr[p, k, n] = A[n, k*128+p] * SCALE
    def a_rhs(M):
        return np.ascontiguousarray(
            (M.T * SCALE).reshape(NC, 128, N).transpose(1, 0, 2)).astype(f8)
    AT8 = a_rhs(A)
    A2T8 = a_rhs(A2)

    def spec_norm(W):
        M = W.reshape(-1, W.shape[-1]).astype(np.float64)
        return W.astype(np.float64) / np.linalg.norm(M, ord=2)

    blk_w = []
    for blk in range(NBLK):
        Wg_n = spec_norm(Wg[blk])
        Wc_n = spec_norm(Wc[blk])
        # bf16 0-hop weights carry 2^10 (gate psum scale; activations apply
        # 2^-10). fp8 hop-1/2 weights are unscaled: inputs arrive as s*2^10.
        PS = 2.0 ** 10
        WG0X = np.zeros((DIN + 1, G), f32)
        WG0X[:DIN] = Wg_n[0][:DIN] * PS
        WG0X[DIN] = bg[blk] * PS
        WG0H = (Wg_n[0][DIN:] * PS).astype(f32)
        WG12 = np.stack([
            np.concatenate([Wg_n[1][:DIN], Wg_n[1][DIN:]], 0),
            np.concatenate([Wg_n[2][:DIN], Wg_n[2][DIN:]], 0)], 1)
        WC0X = np.zeros((DIN + 1, DH), f32)
        WC0X[:DIN] = Wc_n[0][:DIN] * PS
        WC0X[DIN] = bc[blk] * PS
        WC0H = (Wc_n[0][DIN:] * PS).astype(f32)
        WC12X = np.stack([Wc_n[1][:DIN], Wc_n[2][:DIN]], 1)
        WC12H = np.stack([Wc_n[1][DIN:], Wc_n[2][DIN:]], 1)
        blk_w.append({
            "WG0X": WG0X.astype(bf), "WG0H": WG0H.astype(bf),
            "WG12": WG12.astype(f8),
            "WC0X": WC0X.astype(bf), "WC0H": WC0H.astype(bf),
            "WC12X": WC12X.astype(f8), "WC12H": WC12H.astype(f8),
        })

    zeros_xf = np.zeros((128, T, NC, DIN), bf)
    in_maps = []
    for core in range(8):
        b = core % B
        role = core // B  # 0 = block-1 runner, 1 = block-2 runner
        im = dict(blk_w[role])
        im["AT8"] = AT8
        im["A2T8"] = A2T8
        if role == 0:
            im["XF"] = np.ascontiguousarray(
                X[b].reshape(T, NC, 128, DIN).transpose(2, 0, 1, 3)).astype(bf)
        else:
            im["XF"] = zeros_xf
        im["RM"] = np.full((128, 1), float(role), f32)
        im["ZM"] = np.full((128, NC * DH), float(1 - role), f32)
        in_maps.append(im)
    return in_maps


_CACHED = {}


def _get_nc(trace_sim=False, dbg_slot=None):
    key = (trace_sim, dbg_slot)
    if key not in _CACHED:
        _CACHED[key] = build_kernel(trace_sim, dbg_slot)
    return _CACHED[key]


def run_on_device(inputs, dbg_slot=None):
    """Returns per-batch final h [B, N, DH] fp32 (block-2 cores' HOUT)."""
    nc = _get_nc(dbg_slot=dbg_slot)
    in_maps = _prep_inputs(inputs["X"], inputs["A_x"], inputs["Wg"], inputs["bg"],
                           inputs["Wc"], inputs["bc"])
    res = bass_utils.run_bass_kernel_spmd(nc, in_maps, core_ids=list(range(8)))
    results = res.results
    hs = []
    for b in range(B):
        hb = results[4 + b]["HOUT"].reshape(128, NC, DH).transpose(1, 0, 2)
        hs.append(hb.reshape(N, DH))
    return np.stack(hs), results


def kernel(**inputs):
    W_out = inputs["W_out"].astype(np.float64)
    b_out = inputs["b_out"].astype(np.float64)
    hs, _ = run_on_device(inputs)
    W_sn = W_out / np.linalg.norm(W_out)
    pred = hs.astype(np.float64) @ W_sn + b_out     # [B, N, 1]
    out = pred.squeeze(-1).mean()
    return np.float32(out)


if __name__ == "__main__":
    pass


# revision 5
# speedup vs baseline: 5.6364x; 1.0616x over previous
"""Trainium2 Bass kernel for nn_Discriminator (DCRNN-style GRU discriminator), v2.

Design (replaces the node-sharded v1 which serialized 29 x 15us AllGathers):
  - 8 cores = 4 batch-pairs. Core b runs GRU block 1 for batch b over the FULL
    graph; core b+4 runs block 2, software-pipelined SKEW slots behind.
  - Per slot, one small fp8 AllGather per pair hands h1(t) from the block-1
    core to the block-2 core; with SKEW=2 it has a full slot of slack and
    stays off the critical path. SPMD uniformity is kept by masking: both
    roles run identical instructions, A-cores multiply the received payload
    by 0, B-cores read their (host-zeroed) X as 0.
  - All A/A^2 matmuls run fp8-e4m3 with DoubleRow perf mode (2 K-tiles per
    pass). A and A^2 are pre-scaled by 2^17 on host (entries ~1e-3 would be
    subnormal in e4m3); the scale is divided out of the hop-1/2 gate weights.
  - dconv(xh) splits into x-terms and h-terms; the g-path A-matmul packs
    [xin | h] as a 128-wide stationary operand so both sides share one
    stream of A columns.
  - Gate matmuls and transposes in bf16, state/elementwise in fp32.
    Host does the final tiny pred = H[:,-1] @ W_sn + b_out and the mean.
"""
import numpy as np
import ml_dtypes

import concourse.bass as bass
import concourse.mybir as mybir
import concourse.tile as tile
from concourse import bacc
from concourse import bass_utils
from concourse.masks import make_identity

FP32 = mybir.dt.float32
BF16 = mybir.dt.bfloat16
F8 = mybir.dt.float8e4
AF = mybir.ActivationFunctionType
DR = mybir.MatmulPerfMode.DoubleRow

B, T, N, DIN, DH, K, NBLK = 4, 8, 2048, 64, 64, 3, 2
NC = N // 128     # 16 node chunks
G = 2 * DH        # 128 gate width
SKEW = 2
SLOTS = T + SKEW
SCALE = float(2 ** 17)


def build_kernel(trace_sim=False, dbg_slot=None):
    nc = bacc.Bacc(None, target_bir_lowering=False)

    AT8_d = nc.dram_tensor("AT8", [128, NC, N], F8, kind="ExternalInput")
    A2T8_d = nc.dram_tensor("A2T8", [128, NC, N], F8, kind="ExternalInput")
    XF_d = nc.dram_tensor("XF", [128, T, NC, DIN], BF16, kind="ExternalInput")
    RM_d = nc.dram_tensor("RM", [128, 1], FP32, kind="ExternalInput")
    ZM_d = nc.dram_tensor("ZM", [128, NC * DH], FP32, kind="ExternalInput")
    WG0X_d = nc.dram_tensor("WG0X", [DIN + 1, G], BF16, kind="ExternalInput")
    WG0H_d = nc.dram_tensor("WG0H", [DH, G], BF16, kind="ExternalInput")
    WG12_d = nc.dram_tensor("WG12", [128, 2, G], F8, kind="ExternalInput")
    WC0X_d = nc.dram_tensor("WC0X", [DIN + 1, DH], BF16, kind="ExternalInput")
    WC0H_d = nc.dram_tensor("WC0H", [DH, DH], BF16, kind="ExternalInput")
    WC12X_d = nc.dram_tensor("WC12X", [DIN, 2, DH], F8, kind="ExternalInput")
    WC12H_d = nc.dram_tensor("WC12H", [DH, 2, DH], F8, kind="ExternalInput")
    HOUT_d = nc.dram_tensor("HOUT", [128, NC * DH], FP32, kind="ExternalOutput")

    RG = [[b, b + 4] for b in range(B)]

    with tile.TileContext(nc, trace_sim=trace_sim) as tc:
        with (
            tc.tile_pool(name="big", bufs=1) as big,
            tc.tile_pool(name="st", bufs=2) as st,
            tc.tile_pool(name="wk", bufs=2) as wk,
            tc.tile_pool(name="w1", bufs=1) as w1,
            tc.tile_pool(name="psA", bufs=3, space="PSUM") as psA,
            tc.tile_pool(name="psG", bufs=3, space="PSUM") as psG,
            tc.tile_pool(name="psT", bufs=2, space="PSUM") as psT,
            tc.tile_pool(name="dram", bufs=1, space="DRAM") as dram,
        ):
            AT8 = big.tile([128, NC, N], F8)
            A2T8 = big.tile([128, NC, N], F8)
            XF = big.tile([128, T, NC, DIN], BF16)
            RM = big.tile([128, 1], FP32)
            ZM = big.tile([128, NC * DH], FP32)
            nc.sync.dma_start(XF[:, 0:1], XF_d[:, 0:1])
            # split the 8MB of A stripes across the SP and Pool DMA queues so
            # slot 0 is paced at ~2x the single-queue load rate
            for srcd, dstt in ((AT8_d, AT8), (A2T8_d, A2T8)):
                nc.sync.dma_start(dstt[:, :, 0:512], srcd[:, :, 0:512])
            for srcd, dstt in ((AT8_d, AT8), (A2T8_d, A2T8)):
                nc.gpsimd.dma_start(
                    dstt[:, :, 512:1024], srcd[:, :, 512:1024])

            ident32 = big.tile([128, 128], FP32)
            make_identity(nc, ident32[:])
            ident = big.tile([128, 128], BF16)
            nc.vector.tensor_copy(ident[:], ident32[:])
            ones_bf = big.tile([1, N], BF16)
            nc.gpsimd.memset(ones_bf[:], 1.0)

            def wt(dram_t, p, f, nm):
                tl = big.tile([p, f], BF16, name=nm, tag=nm)
                nc.sync.dma_start(tl[:], dram_t[:])
                return tl
            WG0X = wt(WG0X_d, DIN + 1, G, "wg0x")
            WG0H = wt(WG0H_d, DH, G, "wg0h")
            WC0X = wt(WC0X_d, DIN + 1, DH, "wc0x")
            WC0H = wt(WC0H_d, DH, DH, "wc0h")
            WG12 = big.tile([128, 2, G], F8, name="wg12", tag="wg12")
            nc.sync.dma_start(WG12[:], WG12_d[:])
            WC12X = big.tile([DIN, 2, DH], F8, name="wc12x", tag="wc12x")
            nc.sync.dma_start(WC12X[:], WC12X_d[:])
            WC12H = big.tile([DH, 2, DH], F8, name="wc12h", tag="wc12h")
            nc.sync.dma_start(WC12H[:], WC12H_d[:])

            # rest of XF, masks, then remaining A stripes in consumption
            # order, alternating queues
            nc.sync.dma_start(RM[:], RM_d[:])
            nc.sync.dma_start(ZM[:], ZM_d[:])
            for j in (2, 3):
                eng = nc.sync if j == 2 else nc.gpsimd
                for src, dst in ((AT8_d, AT8), (A2T8_d, A2T8)):
                    eng.dma_start(
                        dst[:, :, j * 512:(j + 1) * 512],
                        src[:, :, j * 512:(j + 1) * 512])
            nc.sync.dma_start(XF[:, 1:T], XF_d[:, 1:T])

            AGIN = [dram.tile([N, DH], F8, name=f"agin{i}") for i in range(3)]
            AGOUT = [dram.tile([2 * N, DH], F8, name=f"agout{i}") for i in range(3)]

            h = st.tile([128, NC * DH], FP32, tag="h", name="h_init")
            nc.gpsimd.memset(h[:], 0.0)

            def transpose_fm(src_nm, dst, sfx, dt=BF16):
                # src_nm [128, NC*64] node-major -> dst rows 0:64 feat-major
                # [64, N]; transpose two chunks per pass ([128,128] blocks:
                # rows 0:64 = even chunk's feats, 64:128 = odd chunk's)
                for q in range(2):
                    pt = psT.tile([128, 4 * 128], dt, tag="pt", name="pt" + sfx)
                    for ci in range(4):
                        c2 = q * 4 + ci
                        nc.tensor.transpose(
                            pt[:, ci * 128:(ci + 1) * 128],
                            src_nm[:, c2 * 2 * DH:(c2 + 1) * 2 * DH],
                            ident32[:] if dt is FP32 else ident[:])
                    dv = dst[0:DH, q * 1024:(q + 1) * 1024].rearrange(
                        "f (k two p) -> f k two p", two=2, p=128)
                    ptv = pt[:].rearrange("r (k p) -> r k p", p=128)
                    if q == 0:
                        nc.vector.tensor_copy(dv[:, :, 0], ptv[0:DH])
                        nc.scalar.activation(dv[:, :, 1], ptv[DH:128], AF.Copy)
                    else:
                        nc.scalar.activation(dv[:, :, 0], ptv[0:DH], AF.Copy)
                        nc.vector.tensor_copy(dv[:, :, 1], ptv[DH:128])

            for s in range(SLOTS):
                t = s % T
                sfx = f"_s{s}"

                # --- receive partner payload first (so the SP queue serves it
                # before this slot's 15us+ collective occupies the queue)
                xin = wk.tile([128, NC * DIN], BF16, tag="xin", name="xin" + sfx)
                xin_nm = xin[:].rearrange("p (c f) -> p c f", c=NC)
                if s >= SKEW:
                    recv8 = wk.tile([128, NC, DIN], F8, tag="rcv", name="rcv" + sfx)
                    nc.sync.dma_start(
                        recv8[:],
                        AGOUT[(s - 1) % 3][0:N, :].rearrange("(c p) f -> p c f", c=NC))
                    # xin = role * recv + XF[t]   (role: 0 on block-1 cores)
                    for hq in range(2):
                        cs_ = slice(hq * 8, (hq + 1) * 8)
                        nc.vector.scalar_tensor_tensor(
                            xin_nm[:, cs_], recv8[:, cs_], RM[:], XF[:, t, cs_],
                            op0=mybir.AluOpType.mult, op1=mybir.AluOpType.add)
                else:
                    nc.vector.tensor_copy(xin_nm, XF[:, t])

                if s == SKEW:
                    hm = wk.tile([128, NC * DH], FP32, tag="h", name="hm")
                    nc.vector.tensor_mul(hm[:], h[:], ZM[:])
                    h = hm

                h_nm = h[:].rearrange("p (c f) -> p c f", c=NC)
                LH = wk.tile([128, NC, 128], F8, tag="LH", name="LH" + sfx)
                nc.vector.tensor_copy(LH[:, 0:8, DIN:128], h_nm[:, 0:8])
                nc.vector.tensor_copy(LH[:, 8:NC, DIN:128], h_nm[:, 8:NC])

                if 1 <= s <= T:
                    agv = AGIN[s % 3][:].rearrange("(c p) f -> p c f", c=NC)
                    nc.sync.dma_start(agv[:, 0:8], LH[:, 0:8, DIN:128])
                    nc.sync.dma_start(agv[:, 8:NC], LH[:, 8:NC, DIN:128])
                    nc.gpsimd.collective_compute(
                        "AllGather", mybir.AluOpType.bypass,
                        ins=[AGIN[s % 3][:]], outs=[AGOUT[s % 3][:]],
                        replica_groups=RG)

                nc.vector.tensor_copy(LH[:, 0:8, 0:DIN], xin_nm[:, 0:8])
                nc.vector.tensor_copy(LH[:, 8:NC, 0:DIN], xin_nm[:, 8:NC])
                hbf = wk.tile([128, NC * DH], BF16, tag="hbf", name="hbf" + sfx)
                nc.scalar.activation(hbf[:], h[:], AF.Copy)

                XIN = wk.tile([DIN + 1, N], BF16, tag="XIN", name="XIN" + sfx)
                transpose_fm(xin, XIN, "x" + sfx)
                nc.vector.tensor_copy(XIN[DIN:DIN + 1, :], ones_bf[:])
                HT = wk.tile([DH, N], BF16, tag="HT", name="HT" + sfx)
                transpose_fm(hbf, HT, "h" + sfx)

                # --- packed [xin | h] diffusion + g gates, interleaved per
                # 512-col j-block so gates/sigmoid/rh trail the A stream.
                # Hop-1/2 results live as DoubleRow K-pairs in one fp8 tile,
                # scaled 2^-7 (psum carries 2^17 from A, stored s*2^10).
                P12 = w1.tile([128, 2, N], F8, tag="P12", name="P12" + sfx)
                g = w1.tile([128, NC * G], FP32, tag="g", name="g" + sfx)
                g_nm = g[:].rearrange("p (c f) -> p c f", c=NC)
                t1 = wk.tile([128, NC * DH], FP32, tag="t1", name="t1" + sfx)
                t1_nm = t1[:].rearrange("p (c f) -> p c f", c=NC)
                um1 = wk.tile([128, NC * DH], FP32, tag="um1", name="um1" + sfx)
                um1_nm = um1[:].rearrange("p (c f) -> p c f", c=NC)
                rh = wk.tile([128, NC * DH], FP32, tag="rh", name="rh" + sfx)
                rh_nm = rh[:].rearrange("p (c f) -> p c f", c=NC)
                rh8 = wk.tile([128, NC, DH], F8, tag="rh8", name="rh8" + sfx)
                for j in range(4):
                    js = slice(j * 512, (j + 1) * 512)
                    for hop, Asrc in enumerate((AT8, A2T8)):
                        pa = psA.tile([128, 512], FP32, tag="pa", name="pa" + sfx)
                        for kk in range(8):
                            nc.tensor.matmul(
                                pa[:],
                                LH[:, 2 * kk:2 * kk + 2, :],
                                Asrc[:, 2 * kk:2 * kk + 2, js],
                                start=(kk == 0), stop=(kk == 7),
                                perf_mode=DR)
                        if hop == 0:
                            nc.vector.tensor_scalar_mul(
                                P12[:, 0, js], pa[:], 2.0 ** -7)
                        else:
                            nc.scalar.activation(
                                P12[:, 1, js], pa[:], AF.Copy, scale=2.0 ** -7)
                    psg = psG.tile([128, 512], FP32, tag="psg", name="psg" + sfx)
                    for ci in range(4):
                        c = j * 4 + ci
                        o = psg[:, ci * G:(ci + 1) * G]
                        sl = slice(c * 128, (c + 1) * 128)
                        nc.tensor.matmul(o, XIN[:, sl], WG0X[:], start=True, stop=False)
                        nc.tensor.matmul(o, HT[:, sl], WG0H[:], start=False, stop=False)
                        nc.tensor.matmul(o, P12[:, :, sl], WG12[:], start=False,
                                         stop=True, perf_mode=DR)
                    nc.scalar.activation(g[:, js], psg[:], AF.Sigmoid,
                                         scale=2.0 ** -10)
                    cs = slice(j * 4, (j + 1) * 4)
                    nc.vector.tensor_mul(
                        rh_nm[:, cs], g_nm[:, cs, 0:DH], h_nm[:, cs])
                    nc.vector.tensor_copy(rh8[:, cs], rh_nm[:, cs])
                    # precompute u*h and (u-1) so the post-tanh update is
                    # only two ops: h_new = u*h - (u-1)*cc
                    nc.vector.tensor_mul(
                        t1_nm[:, cs], g_nm[:, cs, DH:G], h_nm[:, cs])
                    nc.vector.tensor_scalar_sub(
                        um1_nm[:, cs], g_nm[:, cs, DH:G], 1.0)
                rhbf = wk.tile([128, NC * DH], BF16, tag="rhbf", name="rhbf" + sfx)
                nc.scalar.activation(rhbf[:], rh[:], AF.Copy)
                RHT = wk.tile([DH, N], BF16, tag="RHT", name="RHT" + sfx)
                transpose_fm(rhbf, RHT, "r" + sfx)

                # --- c diffusion + c gates + h update, interleaved: 2 j-blocks
                # of SR per gate bank, then tanh + update per half
                SR12 = w1.tile([DH, 2, N], F8, tag="SR12", name="SR12" + sfx)
                cc = w1.tile([128, NC * DH], FP32, tag="cc", name="cc" + sfx)
                hmc = wk.tile([128, NC * DH], FP32, tag="hmc", name="hmc" + sfx)
                hmc_nm = hmc[:].rearrange("p (c f) -> p c f", c=NC)
                h_new = st.tile([128, NC * DH], FP32, tag="h", name="hn" + sfx)
                for q in range(2):
                    for jj in range(2):
                        j = q * 2 + jj
                        js = slice(j * 512, (j + 1) * 512)
                        for hop, Asrc in enumerate((AT8, A2T8)):
                            pc = psA.tile([DH, 512], FP32, tag="pa", name="pc" + sfx)
                            for kk in range(8):
                                nc.tensor.matmul(
                                    pc[:],
                                    rh8[:, 2 * kk:2 * kk + 2, :],
                                    Asrc[:, 2 * kk:2 * kk + 2, js],
                                    start=(kk == 0), stop=(kk == 7),
                                    perf_mode=DR)
                            if hop == 0:
                                nc.vector.tensor_scalar_mul(
                                    SR12[:, 0, js], pc[:], 2.0 ** -7)
                            else:
                                nc.scalar.activation(
                                    SR12[:, 1, js], pc[:], AF.Copy,
                                    scale=2.0 ** -7)
                    psc = psG.tile([128, 512], FP32, tag="psg", name="psc" + sfx)
                    for ci in range(8):
                        c = q * 8 + ci
                        o = psc[:, ci * DH:(ci + 1) * DH]
                        sl = slice(c * 128, (c + 1) * 128)
                        nc.tensor.matmul(o, XIN[:, sl], WC0X[:], start=True, stop=False)
                        nc.tensor.matmul(o, RHT[:, sl], WC0H[:], start=False, stop=False)
                        nc.tensor.matmul(o, P12[0:DIN, :, sl], WC12X[:],
                                         start=False, stop=False, perf_mode=DR)
                        nc.tensor.matmul(o, SR12[:, :, sl], WC12H[:],
                                         start=False, stop=True, perf_mode=DR)
                    hs_ = slice(q * 512, (q + 1) * 512)
                    cs_ = slice(q * 8, (q + 1) * 8)
                    nc.scalar.activation(cc[:, hs_], psc[:], AF.Tanh,
                                         scale=2.0 ** -10)
                    eng = nc.vector if q == 0 else nc.gpsimd
                    eng.tensor_mul(hmc[:, hs_], um1[:, hs_], cc[:, hs_])
                    eng.tensor_sub(h_new[:, hs_], t1[:, hs_], hmc[:, hs_])
                h = h_new

                if dbg_slot is not None and s == dbg_slot:
                    break

            nc.sync.dma_start(HOUT_d[:], h[:])

    nc.finalize()
    return nc


# ---------------------------------------------------------------------------
# host-side preparation and execution
# ---------------------------------------------------------------------------

def _prep_inputs(X, A_x, Wg, bg, Wc, bc):
    f32 = np.float32
    bf = ml_dtypes.bfloat16
    f8 = ml_dtypes.float8_e4m3
    A = A_x.astype(np.float64)
    A2 = A @ A

    # rhs layout [128, NC, N]: arr[p, k, n] = A[n, k*128+p] * SCALE
    def a_rhs(M):
        return np.ascontiguousarray(
            (M.T * SCALE).reshape(NC, 128, N).transpose(1, 0, 2)).astype(f8)
    AT8 = a_rhs(A)
    A2T8 = a_rhs(A2)

    def spec_norm(W):
        M = W.reshape(-1, W.shape[-1]).astype(np.float64)
        return W.astype(np.float64) / np.linalg.norm(M, ord=2)

    blk_w = []
    for blk in range(NBLK):
        Wg_n = spec_norm(Wg[blk])
        Wc_n = spec_norm(Wc[blk])
        # bf16 0-hop weights carry 2^10 (gate psum scale; activations apply
        # 2^-10). fp8 hop-1/2 weights are unscaled: inputs arrive as s*2^10.
        PS = 2.0 ** 10
        WG0X = np.zeros((DIN + 1, G), f32)
        WG0X[:DIN] = Wg_n[0][:DIN] * PS
        WG0X[DIN] = bg[blk] * PS
        WG0H = (Wg_n[0][DIN:] * PS).astype(f32)
        WG12 = np.stack([
            np.concatenate([Wg_n[1][:DIN], Wg_n[1][DIN:]], 0),
            np.concatenate([Wg_n[2][:DIN], Wg_n[2][DIN:]], 0)], 1)
        WC0X = np.zeros((DIN + 1, DH), f32)
        WC0X[:DIN] = Wc_n[0][:DIN] * PS
        WC0X[DIN] = bc[blk] * PS
        WC0H = (Wc_n[0][DIN:] * PS).astype(f32)
        WC12X = np.stack([Wc_n[1][:DIN], Wc_n[2][:DIN]], 1)
        WC12H = np.stack([Wc_n[1][DIN:], Wc_n[2][DIN:]], 1)
        blk_w.append({
            "WG0X": WG0X.astype(bf), "WG0H": WG0H.astype(bf),
            "WG12": WG12.astype(f8),
            "WC0X": WC0X.astype(bf), "WC0H": WC0H.astype(bf),
            "WC12X": WC12X.astype(f8), "WC12H": WC12H.astype(f8),
        })

    zeros_xf = np.zeros((128, T, NC, DIN), bf)
    in_maps = []
    for core in range(8):
        b = core % B
        role = core // B  # 0 = block-1 runner, 1 = block-2 runner
        im = dict(blk_w[role])
        im["AT8"] = AT8
        im["A2T8"] = A2T8
        if role == 0:
            im["XF"] = np.ascontiguousarray(
                X[b].reshape(T, NC, 128, DIN).transpose(2, 0, 1, 3)).astype(bf)
        else:
            im["XF"] = zeros_xf
        im["RM"] = np.full((128, 1), float(role), f32)
        im["ZM"] = np.full((128, NC * DH), float(1 - role), f32)
        in_maps.append(im)
    return in_maps


_CACHED = {}


def _get_nc(trace_sim=False, dbg_slot=None):
    key = (trace_sim, dbg_slot)
    if key not in _CACHED:
        _CACHED[key] = build_kernel(trace_sim, dbg_slot)
    return _CACHED[key]


def run_on_device(inputs, dbg_slot=None):
    """Returns per-batch final h [B, N, DH] fp32 (block-2 cores' HOUT)."""
    nc = _get_nc(dbg_slot=dbg_slot)
    in_maps = _prep_inputs(inputs["X"], inputs["A_x"], inputs["Wg"], inputs["bg"],
                           inputs["Wc"], inputs["bc"])
    res = bass_utils.run_bass_kernel_spmd(nc, in_maps, core_ids=list(range(8)))
    results = res.results
    hs = []
    for b in range(B):
        hb = results[4 + b]["HOUT"].reshape(128, NC, DH).transpose(1, 0, 2)
        hs.append(hb.reshape(N, DH))
    return np.stack(hs), results


def kernel(**inputs):
    W_out = inputs["W_out"].astype(np.float64)
    b_out = inputs["b_out"].astype(np.float64)
    hs, _ = run_on_device(inputs)
    W_sn = W_out / np.linalg.norm(W_out)
    pred = hs.astype(np.float64) @ W_sn + b_out     # [B, N, 1]
    out = pred.squeeze(-1).mean()
    return np.float32(out)


if __name__ == "__main__":
    pass


# revision 6
# speedup vs baseline: 5.7484x; 1.0199x over previous
"""Trainium2 Bass kernel for nn_Discriminator (DCRNN-style GRU discriminator), v2.

Design (replaces the node-sharded v1 which serialized 29 x 15us AllGathers):
  - 8 cores = 4 batch-pairs. Core b runs GRU block 1 for batch b over the FULL
    graph; core b+4 runs block 2, software-pipelined SKEW slots behind.
  - Per slot, one small fp8 AllGather per pair hands h1(t) from the block-1
    core to the block-2 core; with SKEW=2 it has a full slot of slack and
    stays off the critical path. SPMD uniformity is kept by masking: both
    roles run identical instructions, A-cores multiply the received payload
    by 0, B-cores read their (host-zeroed) X as 0.
  - All A/A^2 matmuls run fp8-e4m3 with DoubleRow perf mode (2 K-tiles per
    pass). A and A^2 are pre-scaled by 2^17 on host (entries ~1e-3 would be
    subnormal in e4m3); the scale is divided out of the hop-1/2 gate weights.
  - dconv(xh) splits into x-terms and h-terms; the g-path A-matmul packs
    [xin | h] as a 128-wide stationary operand so both sides share one
    stream of A columns.
  - Gate matmuls and transposes in bf16, state/elementwise in fp32.
    Host does the final tiny pred = H[:,-1] @ W_sn + b_out and the mean.
"""
import numpy as np
import ml_dtypes

import concourse.bass as bass
import concourse.mybir as mybir
import concourse.tile as tile
from concourse import bacc
from concourse import bass_utils
from concourse.masks import make_identity

FP32 = mybir.dt.float32
BF16 = mybir.dt.bfloat16
F8 = mybir.dt.float8e4
AF = mybir.ActivationFunctionType
DR = mybir.MatmulPerfMode.DoubleRow

B, T, N, DIN, DH, K, NBLK = 4, 8, 2048, 64, 64, 3, 2
NC = N // 128     # 16 node chunks
G = 2 * DH        # 128 gate width
SKEW = 2
SLOTS = T + SKEW
SCALE = float(2 ** 17)


def build_kernel(trace_sim=False, dbg_slot=None):
    nc = bacc.Bacc(None, target_bir_lowering=False)

    AT8_d = nc.dram_tensor("AT8", [128, NC, N], F8, kind="ExternalInput")
    A2T8_d = nc.dram_tensor("A2T8", [128, NC, N], F8, kind="ExternalInput")
    XF_d = nc.dram_tensor("XF", [128, T, NC, DIN], BF16, kind="ExternalInput")
    RM_d = nc.dram_tensor("RM", [128, 1], FP32, kind="ExternalInput")
    ZM_d = nc.dram_tensor("ZM", [128, NC * DH], FP32, kind="ExternalInput")
    WG0X_d = nc.dram_tensor("WG0X", [DIN + 1, G], BF16, kind="ExternalInput")
    WG0H_d = nc.dram_tensor("WG0H", [DH, G], BF16, kind="ExternalInput")
    WG12_d = nc.dram_tensor("WG12", [128, 2, G], F8, kind="ExternalInput")
    WC0X_d = nc.dram_tensor("WC0X", [DIN + 1, DH], BF16, kind="ExternalInput")
    WC0H_d = nc.dram_tensor("WC0H", [DH, DH], BF16, kind="ExternalInput")
    WC12X_d = nc.dram_tensor("WC12X", [DIN, 2, DH], F8, kind="ExternalInput")
    WC12H_d = nc.dram_tensor("WC12H", [DH, 2, DH], F8, kind="ExternalInput")
    HOUT_d = nc.dram_tensor("HOUT", [128, NC * DH], FP32, kind="ExternalOutput")

    RG = [[b, b + 4] for b in range(B)]

    with tile.TileContext(nc, trace_sim=trace_sim) as tc:
        with (
            tc.tile_pool(name="big", bufs=1) as big,
            tc.tile_pool(name="st", bufs=2) as st,
            tc.tile_pool(name="wk", bufs=2) as wk,
            tc.tile_pool(name="w1", bufs=1) as w1,
            tc.tile_pool(name="psA", bufs=3, space="PSUM") as psA,
            tc.tile_pool(name="psG", bufs=3, space="PSUM") as psG,
            tc.tile_pool(name="psT", bufs=2, space="PSUM") as psT,
            tc.tile_pool(name="dram", bufs=1, space="DRAM") as dram,
        ):
            AT8 = big.tile([128, NC, N], F8)
            A2T8 = big.tile([128, NC, N], F8)
            XF = big.tile([128, T, NC, DIN], BF16)
            RM = big.tile([128, 1], FP32)
            ZM = big.tile([128, NC * DH], FP32)
            nc.sync.dma_start(XF[:, 0:1], XF_d[:, 0:1])
            # split the 8MB of A stripes across the SP, Act and Pool DMA
            # queues so slot 0 is paced at ~3x the single-queue load rate
            q_of = {(0, 0): nc.sync, (0, 1): nc.sync,
                    (1, 0): nc.scalar, (1, 1): nc.scalar,
                    (2, 0): nc.gpsimd, (2, 1): nc.gpsimd,
                    (3, 0): nc.scalar, (3, 1): nc.gpsimd}
            for j in range(4):
                for hop, (srcd, dstt) in enumerate(((AT8_d, AT8), (A2T8_d, A2T8))):
                    q_of[(j, hop)].dma_start(
                        dstt[:, :, j * 512:(j + 1) * 512],
                        srcd[:, :, j * 512:(j + 1) * 512])

            ident32 = big.tile([128, 128], FP32)
            make_identity(nc, ident32[:])
            ident = big.tile([128, 128], BF16)
            nc.vector.tensor_copy(ident[:], ident32[:])
            ones_bf = big.tile([1, N], BF16)
            nc.gpsimd.memset(ones_bf[:], 1.0)

            def wt(dram_t, p, f, nm):
                tl = big.tile([p, f], BF16, name=nm, tag=nm)
                nc.sync.dma_start(tl[:], dram_t[:])
                return tl
            WG0X = wt(WG0X_d, DIN + 1, G, "wg0x")
            WG0H = wt(WG0H_d, DH, G, "wg0h")
            WC0X = wt(WC0X_d, DIN + 1, DH, "wc0x")
            WC0H = wt(WC0H_d, DH, DH, "wc0h")
            WG12 = big.tile([128, 2, G], F8, name="wg12", tag="wg12")
            nc.sync.dma_start(WG12[:], WG12_d[:])
            WC12X = big.tile([DIN, 2, DH], F8, name="wc12x", tag="wc12x")
            nc.sync.dma_start(WC12X[:], WC12X_d[:])
            WC12H = big.tile([DH, 2, DH], F8, name="wc12h", tag="wc12h")
            nc.sync.dma_start(WC12H[:], WC12H_d[:])

            nc.sync.dma_start(RM[:], RM_d[:])
            nc.sync.dma_start(ZM[:], ZM_d[:])
            nc.sync.dma_start(XF[:, 1:T], XF_d[:, 1:T])

            AGIN = [dram.tile([N, DH], F8, name=f"agin{i}") for i in range(3)]
            AGOUT = [dram.tile([2 * N, DH], F8, name=f"agout{i}") for i in range(3)]

            h = st.tile([128, NC * DH], FP32, tag="h", name="h_init")
            nc.gpsimd.memset(h[:], 0.0)

            def transpose_fm(src_nm, dst, sfx, dt=BF16):
                # src_nm [128, NC*64] node-major -> dst rows 0:64 feat-major
                # [64, N]; transpose two chunks per pass ([128,128] blocks:
                # rows 0:64 = even chunk's feats, 64:128 = odd chunk's)
                for q in range(2):
                    pt = psT.tile([128, 4 * 128], dt, tag="pt", name="pt" + sfx)
                    for ci in range(4):
                        c2 = q * 4 + ci
                        nc.tensor.transpose(
                            pt[:, ci * 128:(ci + 1) * 128],
                            src_nm[:, c2 * 2 * DH:(c2 + 1) * 2 * DH],
                            ident32[:] if dt is FP32 else ident[:])
                    dv = dst[0:DH, q * 1024:(q + 1) * 1024].rearrange(
                        "f (k two p) -> f k two p", two=2, p=128)
                    ptv = pt[:].rearrange("r (k p) -> r k p", p=128)
                    if q == 0:
                        nc.vector.tensor_copy(dv[:, :, 0], ptv[0:DH])
                        nc.scalar.activation(dv[:, :, 1], ptv[DH:128], AF.Copy)
                    else:
                        nc.scalar.activation(dv[:, :, 0], ptv[0:DH], AF.Copy)
                        nc.vector.tensor_copy(dv[:, :, 1], ptv[DH:128])

            for s in range(SLOTS):
                t = s % T
                sfx = f"_s{s}"

                # --- receive partner payload first (so the SP queue serves it
                # before this slot's 15us+ collective occupies the queue)
                xin = wk.tile([128, NC * DIN], BF16, tag="xin", name="xin" + sfx)
                xin_nm = xin[:].rearrange("p (c f) -> p c f", c=NC)
                if s >= SKEW:
                    recv8 = wk.tile([128, NC, DIN], F8, tag="rcv", name="rcv" + sfx)
                    nc.sync.dma_start(
                        recv8[:],
                        AGOUT[(s - 1) % 3][0:N, :].rearrange("(c p) f -> p c f", c=NC))
                    # xin = role * recv + XF[t]   (role: 0 on block-1 cores)
                    for hq in range(2):
                        cs_ = slice(hq * 8, (hq + 1) * 8)
                        nc.vector.scalar_tensor_tensor(
                            xin_nm[:, cs_], recv8[:, cs_], RM[:], XF[:, t, cs_],
                            op0=mybir.AluOpType.mult, op1=mybir.AluOpType.add)
                else:
                    nc.vector.tensor_copy(xin_nm, XF[:, t])

                if s == SKEW:
                    hm = wk.tile([128, NC * DH], FP32, tag="h", name="hm")
                    nc.vector.tensor_mul(hm[:], h[:], ZM[:])
                    h = hm

                h_nm = h[:].rearrange("p (c f) -> p c f", c=NC)
                LH = wk.tile([128, NC, 128], F8, tag="LH", name="LH" + sfx)
                nc.vector.tensor_copy(LH[:, 0:8, DIN:128], h_nm[:, 0:8])
                nc.gpsimd.tensor_copy(LH[:, 8:NC, DIN:128], h_nm[:, 8:NC])

                if 1 <= s <= T:
                    agv = AGIN[s % 3][:].rearrange("(c p) f -> p c f", c=NC)
                    nc.sync.dma_start(agv[:, 0:8], LH[:, 0:8, DIN:128])
                    nc.sync.dma_start(agv[:, 8:NC], LH[:, 8:NC, DIN:128])
                    nc.gpsimd.collective_compute(
                        "AllGather", mybir.AluOpType.bypass,
                        ins=[AGIN[s % 3][:]], outs=[AGOUT[s % 3][:]],
                        replica_groups=RG)

                nc.vector.tensor_copy(LH[:, 0:8, 0:DIN], xin_nm[:, 0:8])
                nc.vector.tensor_copy(LH[:, 8:NC, 0:DIN], xin_nm[:, 8:NC])
                hbf = wk.tile([128, NC * DH], BF16, tag="hbf", name="hbf" + sfx)
                nc.scalar.activation(hbf[:], h[:], AF.Copy)

                XIN = wk.tile([DIN + 1, N], BF16, tag="XIN", name="XIN" + sfx)
                transpose_fm(xin, XIN, "x" + sfx)
                nc.vector.tensor_copy(XIN[DIN:DIN + 1, :], ones_bf[:])
                HT = wk.tile([DH, N], BF16, tag="HT", name="HT" + sfx)
                transpose_fm(hbf, HT, "h" + sfx)

                # --- packed [xin | h] diffusion + g gates, interleaved per
                # 512-col j-block so gates/sigmoid/rh trail the A stream.
                # Hop-1/2 results live as DoubleRow K-pairs in one fp8 tile,
                # scaled 2^-7 (psum carries 2^17 from A, stored s*2^10).
                P12 = w1.tile([128, 2, N], F8, tag="P12", name="P12" + sfx)
                g = w1.tile([128, NC * G], FP32, tag="g", name="g" + sfx)
                g_nm = g[:].rearrange("p (c f) -> p c f", c=NC)
                t1 = wk.tile([128, NC * DH], FP32, tag="t1", name="t1" + sfx)
                t1_nm = t1[:].rearrange("p (c f) -> p c f", c=NC)
                um1 = wk.tile([128, NC * DH], FP32, tag="um1", name="um1" + sfx)
                um1_nm = um1[:].rearrange("p (c f) -> p c f", c=NC)
                rh = wk.tile([128, NC * DH], FP32, tag="rh", name="rh" + sfx)
                rh_nm = rh[:].rearrange("p (c f) -> p c f", c=NC)
                rh8 = wk.tile([128, NC, DH], F8, tag="rh8", name="rh8" + sfx)
                for j in range(4):
                    js = slice(j * 512, (j + 1) * 512)
                    for hop, Asrc in enumerate((AT8, A2T8)):
                        pa = psA.tile([128, 512], FP32, tag="pa", name="pa" + sfx)
                        for kk in range(8):
                            nc.tensor.matmul(
                                pa[:],
                                LH[:, 2 * kk:2 * kk + 2, :],
                                Asrc[:, 2 * kk:2 * kk + 2, js],
                                start=(kk == 0), stop=(kk == 7),
                                perf_mode=DR)
                        if hop == 0:
                            nc.vector.tensor_scalar_mul(
                                P12[:, 0, js], pa[:], 2.0 ** -7)
                        else:
                            nc.scalar.activation(
                                P12[:, 1, js], pa[:], AF.Copy, scale=2.0 ** -7)
                    psg = psG.tile([128, 512], FP32, tag="psg", name="psg" + sfx)
                    for ci in range(4):
                        c = j * 4 + ci
                        o = psg[:, ci * G:(ci + 1) * G]
                        sl = slice(c * 128, (c + 1) * 128)
                        nc.tensor.matmul(o, XIN[:, sl], WG0X[:], start=True, stop=False)
                        nc.tensor.matmul(o, HT[:, sl], WG0H[:], start=False, stop=False)
                        nc.tensor.matmul(o, P12[:, :, sl], WG12[:], start=False,
                                         stop=True, perf_mode=DR)
                    nc.scalar.activation(g[:, js], psg[:], AF.Sigmoid,
                                         scale=2.0 ** -10)
                    cs = slice(j * 4, (j + 1) * 4)
                    nc.vector.tensor_mul(
                        rh_nm[:, cs], g_nm[:, cs, 0:DH], h_nm[:, cs])
                    nc.vector.tensor_copy(rh8[:, cs], rh_nm[:, cs])
                    # precompute u*h and (u-1) so the post-tanh update is
                    # only two ops: h_new = u*h - (u-1)*cc
                    nc.vector.tensor_mul(
                        t1_nm[:, cs], g_nm[:, cs, DH:G], h_nm[:, cs])
                    nc.vector.tensor_scalar_sub(
                        um1_nm[:, cs], g_nm[:, cs, DH:G], 1.0)
                rhbf = wk.tile([128, NC * DH], BF16, tag="rhbf", name="rhbf" + sfx)
                nc.scalar.activation(rhbf[:], rh[:], AF.Copy)
                RHT = wk.tile([DH, N], BF16, tag="RHT", name="RHT" + sfx)
                transpose_fm(rhbf, RHT, "r" + sfx)

                # --- c diffusion + c gates + h update, interleaved: 2 j-blocks
                # of SR per gate bank, then tanh + update per half
                SR12 = w1.tile([DH, 2, N], F8, tag="SR12", name="SR12" + sfx)
                cc = w1.tile([128, NC * DH], FP32, tag="cc", name="cc" + sfx)
                hmc = wk.tile([128, NC * DH], FP32, tag="hmc", name="hmc" + sfx)
                hmc_nm = hmc[:].rearrange("p (c f) -> p c f", c=NC)
                h_new = st.tile([128, NC * DH], FP32, tag="h", name="hn" + sfx)
                for q in range(2):
                    for jj in range(2):
                        j = q * 2 + jj
                        js = slice(j * 512, (j + 1) * 512)
                        for hop, Asrc in enumerate((AT8, A2T8)):
                            pc = psA.tile([DH, 512], FP32, tag="pa", name="pc" + sfx)
                            for kk in range(8):
                                nc.tensor.matmul(
                                    pc[:],
                                    rh8[:, 2 * kk:2 * kk + 2, :],
                                    Asrc[:, 2 * kk:2 * kk + 2, js],
                                    start=(kk == 0), stop=(kk == 7),
                                    perf_mode=DR)
                            if hop == 0:
                                nc.vector.tensor_scalar_mul(
                                    SR12[:, 0, js], pc[:], 2.0 ** -7)
                            else:
                                nc.scalar.activation(
                                    SR12[:, 1, js], pc[:], AF.Copy,
                                    scale=2.0 ** -7)
                    psc = psG.tile([128, 512], FP32, tag="psg", name="psc" + sfx)
                    for ci in range(8):
                        c = q * 8 + ci
                        o = psc[:, ci * DH:(ci + 1) * DH]
                        sl = slice(c * 128, (c + 1) * 128)
                        nc.tensor.matmul(o, XIN[:, sl], WC0X[:], start=True, stop=False)
                        nc.tensor.matmul(o, RHT[:, sl], WC0H[:], start=False, stop=False)
                        nc.tensor.matmul(o, P12[0:DIN, :, sl], WC12X[:],
                                         start=False, stop=False, perf_mode=DR)
                        nc.tensor.matmul(o, SR12[:, :, sl], WC12H[:],
                                         start=False, stop=True, perf_mode=DR)
                    hs_ = slice(q * 512, (q + 1) * 512)
                    cs_ = slice(q * 8, (q + 1) * 8)
                    nc.scalar.activation(cc[:, hs_], psc[:], AF.Tanh,
                                         scale=2.0 ** -10)
                    eng = nc.vector if q == 0 else nc.gpsimd
                    eng.tensor_mul(hmc[:, hs_], um1[:, hs_], cc[:, hs_])
                    eng.tensor_sub(h_new[:, hs_], t1[:, hs_], hmc[:, hs_])
                h = h_new

                if dbg_slot is not None and s == dbg_slot:
                    break

            nc.sync.dma_start(HOUT_d[:], h[:])

    nc.finalize()
    return nc


# ---------------------------------------------------------------------------
# host-side preparation and execution
# ---------------------------------------------------------------------------

def _prep_inputs(X, A_x, Wg, bg, Wc, bc):
    f32 = np.float32
    bf = ml_dtypes.bfloat16
    f8 = ml_dtypes.float8_e4m3
    A = A_x.astype(np.float64)
    A2 = A @ A

    # rhs layout [128, NC, N]: arr[p, k, n] = A[n, k*128+p] * SCALE
    def a_rhs(M):
        return np.ascontiguousarray(
            (M.T * SCALE).reshape(NC, 128, N).transpose(1, 0, 2)).astype(f8)
    AT8 = a_rhs(A)
    A2T8 = a_rhs(A2)

    def spec_norm(W):
        M = W.reshape(-1, W.shape[-1]).astype(np.float64)
        return W.astype(np.float64) / np.linalg.norm(M, ord=2)

    blk_w = []
    for blk in range(NBLK):
        Wg_n = spec_norm(Wg[blk])
        Wc_n = spec_norm(Wc[blk])
        # bf16 0-hop weights carry 2^10 (gate psum scale; activations apply
        # 2^-10). fp8 hop-1/2 weights are unscaled: inputs arrive as s*2^10.
        PS = 2.0 ** 10
        WG0X = np.zeros((DIN + 1, G), f32)
        WG0X[:DIN] = Wg_n[0][:DIN] * PS
        WG0X[DIN] = bg[blk] * PS
        WG0H = (Wg_n[0][DIN:] * PS).astype(f32)
        WG12 = np.stack([
            np.concatenate([Wg_n[1][:DIN], Wg_n[1][DIN:]], 0),
            np.concatenate([Wg_n[2][:DIN], Wg_n[2][DIN:]], 0)], 1)
        WC0X = np.zeros((DIN + 1, DH), f32)
        WC0X[:DIN] = Wc_n[0][:DIN] * PS
        WC0X[DIN] = bc[blk] * PS
        WC0H = (Wc_n[0][DIN:] * PS).astype(f32)
        WC12X = np.stack([Wc_n[1][:DIN], Wc_n[2][:DIN]], 1)
        WC12H = np.stack([Wc_n[1][DIN:], Wc_n[2][DIN:]], 1)
        blk_w.append({
            "WG0X": WG0X.astype(bf), "WG0H": WG0H.astype(bf),
            "WG12": WG12.astype(f8),
            "WC0X": WC0X.astype(bf), "WC0H": WC0H.astype(bf),
            "WC12X": WC12X.astype(f8), "WC12H": WC12H.astype(f8),
        })

    zeros_xf = np.zeros((128, T, NC, DIN), bf)
    in_maps = []
    for core in range(8):
        b = core % B
        role = core // B  # 0 = block-1 runner, 1 = block-2 runner
        im = dict(blk_w[role])
        im["AT8"] = AT8
        im["A2T8"] = A2T8
        if role == 0:
            im["XF"] = np.ascontiguousarray(
                X[b].reshape(T, NC, 128, DIN).transpose(2, 0, 1, 3)).astype(bf)
        else:
            im["XF"] = zeros_xf
        im["RM"] = np.full((128, 1), float(role), f32)
        im["ZM"] = np.full((128, NC * DH), float(1 - role), f32)
        in_maps.append(im)
    return in_maps


_CACHED = {}


def _get_nc(trace_sim=False, dbg_slot=None):
    key = (trace_sim, dbg_slot)
    if key not in _CACHED:
        _CACHED[key] = build_kernel(trace_sim, dbg_slot)
    return _CACHED[key]


def run_on_device(inputs, dbg_slot=None):
    """Returns per-batch final h [B, N, DH] fp32 (block-2 cores' HOUT)."""
    nc = _get_nc(dbg_slot=dbg_slot)
    in_maps = _prep_inputs(inputs["X"], inputs["A_x"], inputs["Wg"], inputs["bg"],
                           inputs["Wc"], inputs["bc"])
    res = bass_utils.run_bass_kernel_spmd(nc, in_maps, core_ids=list(range(8)))
    results = res.results
    hs = []
    for b in range(B):
        hb = results[4 + b]["HOUT"].reshape(128, NC, DH).transpose(1, 0, 2)
        hs.append(hb.reshape(N, DH))
    return np.stack(hs), results


def kernel(**inputs):
    W_out = inputs["W_out"].astype(np.float64)
    b_out = inputs["b_out"].astype(np.float64)
    hs, _ = run_on_device(inputs)
    W_sn = W_out / np.linalg.norm(W_out)
    pred = hs.astype(np.float64) @ W_sn + b_out     # [B, N, 1]
    out = pred.squeeze(-1).mean()
    return np.float32(out)


if __name__ == "__main__":
    pass


# revision 7
# speedup vs baseline: 5.7675x; 1.0033x over previous
"""Trainium2 Bass kernel for nn_Discriminator (DCRNN-style GRU discriminator), v2.

Design (replaces the node-sharded v1 which serialized 29 x 15us AllGathers):
  - 8 cores = 4 batch-pairs. Core b runs GRU block 1 for batch b over the FULL
    graph; core b+4 runs block 2, software-pipelined SKEW slots behind.
  - Per slot, one small fp8 AllGather per pair hands h1(t) from the block-1
    core to the block-2 core; with SKEW=2 it has a full slot of slack and
    stays off the critical path. SPMD uniformity is kept by masking: both
    roles run identical instructions, A-cores multiply the received payload
    by 0, B-cores read their (host-zeroed) X as 0.
  - All A/A^2 matmuls run fp8-e4m3 with DoubleRow perf mode (2 K-tiles per
    pass). A and A^2 are pre-scaled by 2^17 on host (entries ~1e-3 would be
    subnormal in e4m3); the scale is divided out of the hop-1/2 gate weights.
  - dconv(xh) splits into x-terms and h-terms; the g-path A-matmul packs
    [xin | h] as a 128-wide stationary operand so both sides share one
    stream of A columns.
  - Gate matmuls and transposes in bf16, state/elementwise in fp32.
    Host does the final tiny pred = H[:,-1] @ W_sn + b_out and the mean.
"""
import numpy as np
import ml_dtypes

import concourse.bass as bass
import concourse.mybir as mybir
import concourse.tile as tile
from concourse import bacc
from concourse import bass_utils
from concourse.masks import make_identity

FP32 = mybir.dt.float32
BF16 = mybir.dt.bfloat16
F8 = mybir.dt.float8e4
AF = mybir.ActivationFunctionType
DR = mybir.MatmulPerfMode.DoubleRow

B, T, N, DIN, DH, K, NBLK = 4, 8, 2048, 64, 64, 3, 2
NC = N // 128     # 16 node chunks
G = 2 * DH        # 128 gate width
SKEW = 2
SLOTS = T + SKEW
SCALE = float(2 ** 17)


def build_kernel(trace_sim=False, dbg_slot=None):
    nc = bacc.Bacc(None, target_bir_lowering=False)

    AT8_d = nc.dram_tensor("AT8", [128, NC, N], F8, kind="ExternalInput")
    A2T8_d = nc.dram_tensor("A2T8", [128, NC, N], F8, kind="ExternalInput")
    XF_d = nc.dram_tensor("XF", [128, T, NC, DIN], BF16, kind="ExternalInput")
    RM_d = nc.dram_tensor("RM", [128, 1], FP32, kind="ExternalInput")
    ZM_d = nc.dram_tensor("ZM", [128, NC * DH], FP32, kind="ExternalInput")
    WG0X_d = nc.dram_tensor("WG0X", [DIN + 1, G], BF16, kind="ExternalInput")
    WG0H_d = nc.dram_tensor("WG0H", [DH, G], BF16, kind="ExternalInput")
    WG12_d = nc.dram_tensor("WG12", [128, 2, G], F8, kind="ExternalInput")
    WC0X_d = nc.dram_tensor("WC0X", [DIN + 1, DH], BF16, kind="ExternalInput")
    WC0H_d = nc.dram_tensor("WC0H", [DH, DH], BF16, kind="ExternalInput")
    WC12X_d = nc.dram_tensor("WC12X", [DIN, 2, DH], F8, kind="ExternalInput")
    WC12H_d = nc.dram_tensor("WC12H", [DH, 2, DH], F8, kind="ExternalInput")
    HOUT_d = nc.dram_tensor("HOUT", [128, NC * DH], FP32, kind="ExternalOutput")

    RG = [[b, b + 4] for b in range(B)]

    with tile.TileContext(nc, trace_sim=trace_sim) as tc:
        with (
            tc.tile_pool(name="big", bufs=1) as big,
            tc.tile_pool(name="st", bufs=2) as st,
            tc.tile_pool(name="wk", bufs=2) as wk,
            tc.tile_pool(name="w1", bufs=1) as w1,
            tc.tile_pool(name="psA", bufs=3, space="PSUM") as psA,
            tc.tile_pool(name="psG", bufs=3, space="PSUM") as psG,
            tc.tile_pool(name="psT", bufs=2, space="PSUM") as psT,
            tc.tile_pool(name="dram", bufs=1, space="DRAM") as dram,
        ):
            AT8 = big.tile([128, NC, N], F8)
            A2T8 = big.tile([128, NC, N], F8)
            XF = big.tile([128, T, NC, DIN], BF16)
            RM = big.tile([128, 1], FP32)
            ZM = big.tile([128, NC * DH], FP32)
            nc.sync.dma_start(XF[:, 0:1], XF_d[:, 0:1])
            # split the 8MB of A stripes across the SP, Act and Pool DMA
            # queues so slot 0 is paced at ~3x the single-queue load rate
            q_of = {(0, 0): nc.sync, (0, 1): nc.sync,
                    (1, 0): nc.scalar, (1, 1): nc.scalar,
                    (2, 0): nc.gpsimd, (2, 1): nc.gpsimd,
                    (3, 0): nc.scalar, (3, 1): nc.gpsimd}
            for j in range(4):
                for hop, (srcd, dstt) in enumerate(((AT8_d, AT8), (A2T8_d, A2T8))):
                    q_of[(j, hop)].dma_start(
                        dstt[:, :, j * 512:(j + 1) * 512],
                        srcd[:, :, j * 512:(j + 1) * 512])

            ident32 = big.tile([128, 128], FP32)
            make_identity(nc, ident32[:])
            ident = big.tile([128, 128], BF16)
            nc.vector.tensor_copy(ident[:], ident32[:])
            ones_bf = big.tile([1, N], BF16)
            nc.gpsimd.memset(ones_bf[:], 1.0)

            def wt(dram_t, p, f, nm):
                tl = big.tile([p, f], BF16, name=nm, tag=nm)
                nc.sync.dma_start(tl[:], dram_t[:])
                return tl
            WG0X = wt(WG0X_d, DIN + 1, G, "wg0x")
            WG0H = wt(WG0H_d, DH, G, "wg0h")
            WC0X = wt(WC0X_d, DIN + 1, DH, "wc0x")
            WC0H = wt(WC0H_d, DH, DH, "wc0h")
            WG12 = big.tile([128, 2, G], F8, name="wg12", tag="wg12")
            nc.sync.dma_start(WG12[:], WG12_d[:])
            WC12X = big.tile([DIN, 2, DH], F8, name="wc12x", tag="wc12x")
            nc.sync.dma_start(WC12X[:], WC12X_d[:])
            WC12H = big.tile([DH, 2, DH], F8, name="wc12h", tag="wc12h")
            nc.sync.dma_start(WC12H[:], WC12H_d[:])

            nc.sync.dma_start(RM[:], RM_d[:])
            nc.sync.dma_start(ZM[:], ZM_d[:])
            nc.sync.dma_start(XF[:, 1:T], XF_d[:, 1:T])

            AGIN = [dram.tile([N, DH], F8, name=f"agin{i}") for i in range(3)]
            AGOUT = [dram.tile([2 * N, DH], F8, name=f"agout{i}") for i in range(3)]

            h = st.tile([128, NC * DH], FP32, tag="h", name="h_init")
            nc.gpsimd.memset(h[:], 0.0)

            def transpose_fm(src_nm, dst, sfx, dt=BF16):
                # src_nm [128, NC*64] node-major -> dst rows 0:64 feat-major
                # [64, N]; transpose two chunks per pass ([128,128] blocks:
                # rows 0:64 = even chunk's feats, 64:128 = odd chunk's)
                for q in range(2):
                    pt = psT.tile([128, 4 * 128], dt, tag="pt", name="pt" + sfx)
                    for ci in range(4):
                        c2 = q * 4 + ci
                        nc.tensor.transpose(
                            pt[:, ci * 128:(ci + 1) * 128],
                            src_nm[:, c2 * 2 * DH:(c2 + 1) * 2 * DH],
                            ident32[:] if dt is FP32 else ident[:])
                    dv = dst[0:DH, q * 1024:(q + 1) * 1024].rearrange(
                        "f (k two p) -> f k two p", two=2, p=128)
                    ptv = pt[:].rearrange("r (k p) -> r k p", p=128)
                    if q == 0:
                        nc.vector.tensor_copy(dv[:, :, 0], ptv[0:DH])
                        nc.scalar.activation(dv[:, :, 1], ptv[DH:128], AF.Copy)
                    else:
                        nc.scalar.activation(dv[:, :, 0], ptv[0:DH], AF.Copy)
                        nc.vector.tensor_copy(dv[:, :, 1], ptv[DH:128])

            for s in range(SLOTS):
                t = s % T
                sfx = f"_s{s}"

                # --- receive partner payload first (so the SP queue serves it
                # before this slot's 15us+ collective occupies the queue)
                xin = wk.tile([128, NC * DIN], BF16, tag="xin", name="xin" + sfx)
                xin_nm = xin[:].rearrange("p (c f) -> p c f", c=NC)
                if s >= SKEW:
                    recv8 = wk.tile([128, NC, DIN], F8, tag="rcv", name="rcv" + sfx)
                    nc.sync.dma_start(
                        recv8[:],
                        AGOUT[(s - 1) % 3][0:N, :].rearrange("(c p) f -> p c f", c=NC))
                    # xin = role * recv + XF[t]   (role: 0 on block-1 cores)
                    for hq in range(2):
                        cs_ = slice(hq * 8, (hq + 1) * 8)
                        nc.vector.scalar_tensor_tensor(
                            xin_nm[:, cs_], recv8[:, cs_], RM[:], XF[:, t, cs_],
                            op0=mybir.AluOpType.mult, op1=mybir.AluOpType.add)
                else:
                    nc.vector.tensor_copy(xin_nm, XF[:, t])

                if s == SKEW:
                    hm = wk.tile([128, NC * DH], FP32, tag="h", name="hm")
                    nc.vector.tensor_mul(hm[:], h[:], ZM[:])
                    h = hm

                h_nm = h[:].rearrange("p (c f) -> p c f", c=NC)
                LH = wk.tile([128, NC, 128], F8, tag="LH", name="LH" + sfx)
                nc.vector.tensor_copy(LH[:, 0:8, DIN:128], h_nm[:, 0:8])
                nc.gpsimd.tensor_copy(LH[:, 8:NC, DIN:128], h_nm[:, 8:NC])

                if 1 <= s <= T:
                    agv = AGIN[s % 3][:].rearrange("(c p) f -> p c f", c=NC)
                    nc.sync.dma_start(agv[:, 0:8], LH[:, 0:8, DIN:128])
                    nc.sync.dma_start(agv[:, 8:NC], LH[:, 8:NC, DIN:128])
                    nc.gpsimd.collective_compute(
                        "AllGather", mybir.AluOpType.bypass,
                        ins=[AGIN[s % 3][:]], outs=[AGOUT[s % 3][:]],
                        replica_groups=RG)

                nc.vector.tensor_copy(LH[:, 0:8, 0:DIN], xin_nm[:, 0:8])
                nc.vector.tensor_copy(LH[:, 8:NC, 0:DIN], xin_nm[:, 8:NC])
                hbf = wk.tile([128, NC * DH], BF16, tag="hbf", name="hbf" + sfx)
                nc.scalar.activation(hbf[:], h[:], AF.Copy)

                XIN = wk.tile([DIN + 1, N], BF16, tag="XIN", name="XIN" + sfx)
                transpose_fm(xin, XIN, "x" + sfx)
                nc.vector.tensor_copy(XIN[DIN:DIN + 1, :], ones_bf[:])
                HT = wk.tile([DH, N], BF16, tag="HT", name="HT" + sfx)
                transpose_fm(hbf, HT, "h" + sfx)

                # --- packed [xin | h] diffusion + g gates, interleaved per
                # 512-col j-block so gates/sigmoid/rh trail the A stream.
                # Hop-1/2 results live as DoubleRow K-pairs in one fp8 tile,
                # scaled 2^-7 (psum carries 2^17 from A, stored s*2^10).
                P12 = w1.tile([128, 2, N], F8, tag="P12", name="P12" + sfx)
                g = w1.tile([128, NC * G], FP32, tag="g", name="g" + sfx)
                g_nm = g[:].rearrange("p (c f) -> p c f", c=NC)
                t1 = wk.tile([128, NC * DH], FP32, tag="t1", name="t1" + sfx)
                t1_nm = t1[:].rearrange("p (c f) -> p c f", c=NC)
                um1 = wk.tile([128, NC * DH], FP32, tag="um1", name="um1" + sfx)
                um1_nm = um1[:].rearrange("p (c f) -> p c f", c=NC)
                rh = wk.tile([128, NC * DH], FP32, tag="rh", name="rh" + sfx)
                rh_nm = rh[:].rearrange("p (c f) -> p c f", c=NC)
                rh8 = wk.tile([128, NC, DH], F8, tag="rh8", name="rh8" + sfx)
                for j in range(4):
                    js = slice(j * 512, (j + 1) * 512)
                    for hop, Asrc in enumerate((AT8, A2T8)):
                        pa = psA.tile([128, 512], FP32, tag="pa", name="pa" + sfx)
                        for kk in range(8):
                            nc.tensor.matmul(
                                pa[:],
                                LH[:, 2 * kk:2 * kk + 2, :],
                                Asrc[:, 2 * kk:2 * kk + 2, js],
                                start=(kk == 0), stop=(kk == 7),
                                perf_mode=DR)
                        if hop == 0:
                            nc.vector.tensor_scalar_mul(
                                P12[:, 0, js], pa[:], 2.0 ** -7)
                        else:
                            nc.scalar.activation(
                                P12[:, 1, js], pa[:], AF.Copy, scale=2.0 ** -7)
                    psg = psG.tile([128, 512], FP32, tag="psg", name="psg" + sfx)
                    for ci in range(4):
                        c = j * 4 + ci
                        o = psg[:, ci * G:(ci + 1) * G]
                        sl = slice(c * 128, (c + 1) * 128)
                        nc.tensor.matmul(o, XIN[:, sl], WG0X[:], start=True, stop=False)
                        nc.tensor.matmul(o, HT[:, sl], WG0H[:], start=False, stop=False)
                        nc.tensor.matmul(o, P12[:, :, sl], WG12[:], start=False,
                                         stop=True, perf_mode=DR)
                    nc.scalar.activation(g[:, js], psg[:], AF.Sigmoid,
                                         scale=2.0 ** -10)
                    cs = slice(j * 4, (j + 1) * 4)
                    nc.vector.tensor_mul(
                        rh_nm[:, cs], g_nm[:, cs, 0:DH], h_nm[:, cs])
                    nc.vector.tensor_copy(rh8[:, cs], rh_nm[:, cs])
                    # precompute u*h and (u-1) so the post-tanh update is
                    # only two ops: h_new = u*h - (u-1)*cc
                    nc.vector.tensor_mul(
                        t1_nm[:, cs], g_nm[:, cs, DH:G], h_nm[:, cs])
                    nc.vector.tensor_scalar_sub(
                        um1_nm[:, cs], g_nm[:, cs, DH:G], 1.0)
                rhbf = wk.tile([128, NC * DH], BF16, tag="rhbf", name="rhbf" + sfx)
                nc.scalar.activation(rhbf[:], rh[:], AF.Copy)
                RHT = wk.tile([DH, N], BF16, tag="RHT", name="RHT" + sfx)
                transpose_fm(rhbf, RHT, "r" + sfx)

                # --- c diffusion + c gates + h update, interleaved: 2 j-blocks
                # of SR per gate bank, then tanh + update per half
                SR12 = w1.tile([DH, 2, N], F8, tag="SR12", name="SR12" + sfx)
                cc = w1.tile([128, NC * DH], FP32, tag="cc", name="cc" + sfx)
                hmc = wk.tile([128, NC * DH], FP32, tag="hmc", name="hmc" + sfx)
                hmc_nm = hmc[:].rearrange("p (c f) -> p c f", c=NC)
                h_new = st.tile([128, NC * DH], FP32, tag="h", name="hn" + sfx)
                for q in range(2):
                    for jj in range(2):
                        j = q * 2 + jj
                        js = slice(j * 512, (j + 1) * 512)
                        for hop, Asrc in enumerate((AT8, A2T8)):
                            pc = psA.tile([DH, 512], FP32, tag="pa", name="pc" + sfx)
                            for kk in range(8):
                                nc.tensor.matmul(
                                    pc[:],
                                    rh8[:, 2 * kk:2 * kk + 2, :],
                                    Asrc[:, 2 * kk:2 * kk + 2, js],
                                    start=(kk == 0), stop=(kk == 7),
                                    perf_mode=DR)
                            if hop == 0:
                                nc.vector.tensor_scalar_mul(
                                    SR12[:, 0, js], pc[:], 2.0 ** -7)
                            else:
                                nc.scalar.activation(
                                    SR12[:, 1, js], pc[:], AF.Copy,
                                    scale=2.0 ** -7)
                    psc = psG.tile([128, 512], FP32, tag="psg", name="psc" + sfx)
                    for ci in range(8):
                        c = q * 8 + ci
                        o = psc[:, ci * DH:(ci + 1) * DH]
                        sl = slice(c * 128, (c + 1) * 128)
                        nc.tensor.matmul(o, XIN[:, sl], WC0X[:], start=True, stop=False)
                        nc.tensor.matmul(o, RHT[:, sl], WC0H[:], start=False, stop=False)
                        nc.tensor.matmul(o, P12[0:DIN, :, sl], WC12X[:],
                                         start=False, stop=False, perf_mode=DR)
                        nc.tensor.matmul(o, SR12[:, :, sl], WC12H[:],
                                         start=False, stop=True, perf_mode=DR)
                    hs_ = slice(q * 512, (q + 1) * 512)
                    cs_ = slice(q * 8, (q + 1) * 8)
                    nc.scalar.activation(cc[:, hs_], psc[:], AF.Tanh,
                                         scale=2.0 ** -10)
                    eng = nc.vector if q == 0 else nc.gpsimd
                    eng.tensor_mul(hmc[:, hs_], um1[:, hs_], cc[:, hs_])
                    eng.tensor_sub(h_new[:, hs_], t1[:, hs_], hmc[:, hs_])
                h = h_new

                if dbg_slot is not None and s == dbg_slot:
                    break

            nc.sync.dma_start(HOUT_d[:, 0:NC * DH // 2], h[:, 0:NC * DH // 2])
            nc.sync.dma_start(HOUT_d[:, NC * DH // 2:], h[:, NC * DH // 2:])

    nc.finalize()
    return nc


# ---------------------------------------------------------------------------
# host-side preparation and execution
# ---------------------------------------------------------------------------

def _prep_inputs(X, A_x, Wg, bg, Wc, bc):
    f32 = np.float32
    bf = ml_dtypes.bfloat16
    f8 = ml_dtypes.float8_e4m3
    A = A_x.astype(np.float64)
    A2 = A @ A

    # rhs layout [128, NC, N]: arr[p, k, n] = A[n, k*128+p] * SCALE
    def a_rhs(M):
        return np.ascontiguousarray(
            (M.T * SCALE).reshape(NC, 128, N).transpose(1, 0, 2)).astype(f8)
    AT8 = a_rhs(A)
    A2T8 = a_rhs(A2)

    def spec_norm(W):
        M = W.reshape(-1, W.shape[-1]).astype(np.float64)
        return W.astype(np.float64) / np.linalg.norm(M, ord=2)

    blk_w = []
    for blk in range(NBLK):
        Wg_n = spec_norm(Wg[blk])
        Wc_n = spec_norm(Wc[blk])
        # bf16 0-hop weights carry 2^10 (gate psum scale; activations apply
        # 2^-10). fp8 hop-1/2 weights are unscaled: inputs arrive as s*2^10.
        PS = 2.0 ** 10
        WG0X = np.zeros((DIN + 1, G), f32)
        WG0X[:DIN] = Wg_n[0][:DIN] * PS
        WG0X[DIN] = bg[blk] * PS
        WG0H = (Wg_n[0][DIN:] * PS).astype(f32)
        WG12 = np.stack([
            np.concatenate([Wg_n[1][:DIN], Wg_n[1][DIN:]], 0),
            np.concatenate([Wg_n[2][:DIN], Wg_n[2][DIN:]], 0)], 1)
        WC0X = np.zeros((DIN + 1, DH), f32)
        WC0X[:DIN] = Wc_n[0][:DIN] * PS
        WC0X[DIN] = bc[blk] * PS
        WC0H = (Wc_n[0][DIN:] * PS).astype(f32)
        WC12X = np.stack([Wc_n[1][:DIN], Wc_n[2][:DIN]], 1)
        WC12H = np.stack([Wc_n[1][DIN:], Wc_n[2][DIN:]], 1)
        blk_w.append({
            "WG0X": WG0X.astype(bf), "WG0H": WG0H.astype(bf),
            "WG12": WG12.astype(f8),
            "WC0X": WC0X.astype(bf), "WC0H": WC0H.astype(bf),
            "WC12X": WC12X.astype(f8), "WC12H": WC12H.astype(f8),
        })

    zeros_xf = np.zeros((128, T, NC, DIN), bf)
    in_maps = []
    for core in range(8):
        b = core % B
        role = core // B  # 0 = block-1 runner, 1 = block-2 runner
        im = dict(blk_w[role])
        im["AT8"] = AT8
        im["A2T8"] = A2T8
        if role == 0:
            im["XF"] = np.ascontiguousarray(
                X[b].reshape(T, NC, 128, DIN).transpose(2, 0, 1, 3)).astype(bf)
        else:
            im["XF"] = zeros_xf
        im["RM"] = np.full((128, 1), float(role), f32)
        im["ZM"] = np.full((128, NC * DH), float(1 - role), f32)
        in_maps.append(im)
    return in_maps


_CACHED = {}


def _get_nc(trace_sim=False, dbg_slot=None):
    key = (trace_sim, dbg_slot)
    if key not in _CACHED:
        _CACHED[key] = build_kernel(trace_sim, dbg_slot)
    return _CACHED[key]


def run_on_device(inputs, dbg_slot=None):
    """Returns per-batch final h [B, N, DH] fp32 (block-2 cores' HOUT)."""
    nc = _get_nc(dbg_slot=dbg_slot)
    in_maps = _prep_inputs(inputs["X"], inputs["A_x"], inputs["Wg"], inputs["bg"],
                           inputs["Wc"], inputs["bc"])
    res = bass_utils.run_bass_kernel_spmd(nc, in_maps, core_ids=list(range(8)))
    results = res.results
    hs = []
    for b in range(B):
        hb = results[4 + b]["HOUT"].reshape(128, NC, DH).transpose(1, 0, 2)
        hs.append(hb.reshape(N, DH))
    return np.stack(hs), results


def kernel(**inputs):
    W_out = inputs["W_out"].astype(np.float64)
    b_out = inputs["b_out"].astype(np.float64)
    hs, _ = run_on_device(inputs)
    W_sn = W_out / np.linalg.norm(W_out)
    pred = hs.astype(np.float64) @ W_sn + b_out     # [B, N, 1]
    out = pred.squeeze(-1).mean()
    return np.float32(out)


if __name__ == "__main__":
    pass


# revision 9
# speedup vs baseline: 5.8448x; 1.0134x over previous
"""Trainium2 Bass kernel for nn_Discriminator (DCRNN-style GRU discriminator), v2.

Design (replaces the node-sharded v1 which serialized 29 x 15us AllGathers):
  - 8 cores = 4 batch-pairs. Core b runs GRU block 1 for batch b over the FULL
    graph; core b+4 runs block 2, software-pipelined SKEW slots behind.
  - Per slot, one small fp8 AllGather per pair hands h1(t) from the block-1
    core to the block-2 core; with SKEW=2 it has a full slot of slack and
    stays off the critical path. SPMD uniformity is kept by masking: both
    roles run identical instructions, A-cores multiply the received payload
    by 0, B-cores read their (host-zeroed) X as 0.
  - All A/A^2 matmuls run fp8-e4m3 with DoubleRow perf mode (2 K-tiles per
    pass). A and A^2 are pre-scaled by 2^17 on host (entries ~1e-3 would be
    subnormal in e4m3); the scale is divided out of the hop-1/2 gate weights.
  - dconv(xh) splits into x-terms and h-terms; the g-path A-matmul packs
    [xin | h] as a 128-wide stationary operand so both sides share one
    stream of A columns.
  - Gate matmuls and transposes in bf16, state/elementwise in fp32.
    Host does the final tiny pred = H[:,-1] @ W_sn + b_out and the mean.
"""
import numpy as np
import ml_dtypes

import concourse.bass as bass
import concourse.mybir as mybir
import concourse.tile as tile
from concourse import bacc
from concourse import bass_utils
from concourse.masks import make_identity

FP32 = mybir.dt.float32
BF16 = mybir.dt.bfloat16
F8 = mybir.dt.float8e4
AF = mybir.ActivationFunctionType
DR = mybir.MatmulPerfMode.DoubleRow

B, T, N, DIN, DH, K, NBLK = 4, 8, 2048, 64, 64, 3, 2
NC = N // 128     # 16 node chunks
G = 2 * DH        # 128 gate width
SKEW = 2
SLOTS = T + SKEW
SCALE = float(2 ** 17)


def build_kernel(trace_sim=False, dbg_slot=None):
    nc = bacc.Bacc(None, target_bir_lowering=False)

    AT8_d = nc.dram_tensor("AT8", [128, NC, N], F8, kind="ExternalInput")
    A2T8_d = nc.dram_tensor("A2T8", [128, NC, N], F8, kind="ExternalInput")
    XF8_d = nc.dram_tensor("XF8", [128, T, NC, DIN], F8, kind="ExternalInput")
    RM_d = nc.dram_tensor("RM", [128, 1], FP32, kind="ExternalInput")
    ZM_d = nc.dram_tensor("ZM", [128, NC * DH], FP32, kind="ExternalInput")
    WG0X_d = nc.dram_tensor("WG0X", [DIN + 1, G], BF16, kind="ExternalInput")
    WG0H_d = nc.dram_tensor("WG0H", [DH, G], BF16, kind="ExternalInput")
    WG12_d = nc.dram_tensor("WG12", [128, 2, G], F8, kind="ExternalInput")
    WC0X_d = nc.dram_tensor("WC0X", [DIN + 1, DH], BF16, kind="ExternalInput")
    WC0H_d = nc.dram_tensor("WC0H", [DH, DH], BF16, kind="ExternalInput")
    WC12X_d = nc.dram_tensor("WC12X", [DIN, 2, DH], F8, kind="ExternalInput")
    WC12H_d = nc.dram_tensor("WC12H", [DH, 2, DH], F8, kind="ExternalInput")
    HOUT_d = nc.dram_tensor("HOUT", [128, NC * DH], FP32, kind="ExternalOutput")

    RG = [[b, b + 4] for b in range(B)]

    with tile.TileContext(nc, trace_sim=trace_sim) as tc:
        with (
            tc.tile_pool(name="big", bufs=1) as big,
            tc.tile_pool(name="st", bufs=2) as st,
            tc.tile_pool(name="wk", bufs=2) as wk,
            tc.tile_pool(name="w1", bufs=1) as w1,
            tc.tile_pool(name="psA", bufs=3, space="PSUM") as psA,
            tc.tile_pool(name="psG", bufs=3, space="PSUM") as psG,
            tc.tile_pool(name="psT", bufs=2, space="PSUM") as psT,
            tc.tile_pool(name="dram", bufs=1, space="DRAM") as dram,
        ):
            AT8 = big.tile([128, NC, N], F8)
            A2T8 = big.tile([128, NC, N], F8)
            XF8 = big.tile([128, T, NC, DIN], F8, name="xf8", tag="xf8")
            RM = big.tile([128, 1], FP32)
            ZM = big.tile([128, NC * DH], FP32)
            nc.sync.dma_start(XF8[:, 0:1], XF8_d[:, 0:1])
            # split the 8MB of A stripes across the SP, Act and Pool DMA
            # queues so slot 0 is paced at ~3x the single-queue load rate
            q_of = {(0, 0): nc.sync, (0, 1): nc.sync,
                    (1, 0): nc.scalar, (1, 1): nc.scalar,
                    (2, 0): nc.gpsimd, (2, 1): nc.gpsimd,
                    (3, 0): nc.scalar, (3, 1): nc.gpsimd}
            for j in range(4):
                for hop, (srcd, dstt) in enumerate(((AT8_d, AT8), (A2T8_d, A2T8))):
                    q_of[(j, hop)].dma_start(
                        dstt[:, :, j * 512:(j + 1) * 512],
                        srcd[:, :, j * 512:(j + 1) * 512])

            ident32 = big.tile([128, 128], FP32)
            make_identity(nc, ident32[:])
            ident = big.tile([128, 128], BF16)
            nc.vector.tensor_copy(ident[:], ident32[:])
            ones_bf = big.tile([1, N], BF16)
            nc.gpsimd.memset(ones_bf[:], 1.0)

            def wt(dram_t, p, f, nm):
                tl = big.tile([p, f], BF16, name=nm, tag=nm)
                nc.sync.dma_start(tl[:], dram_t[:])
                return tl
            WG0X = wt(WG0X_d, DIN + 1, G, "wg0x")
            WG0H = wt(WG0H_d, DH, G, "wg0h")
            WC0X = wt(WC0X_d, DIN + 1, DH, "wc0x")
            WC0H = wt(WC0H_d, DH, DH, "wc0h")
            WG12 = big.tile([128, 2, G], F8, name="wg12", tag="wg12")
            nc.sync.dma_start(WG12[:], WG12_d[:])
            WC12X = big.tile([DIN, 2, DH], F8, name="wc12x", tag="wc12x")
            nc.sync.dma_start(WC12X[:], WC12X_d[:])
            WC12H = big.tile([DH, 2, DH], F8, name="wc12h", tag="wc12h")
            nc.sync.dma_start(WC12H[:], WC12H_d[:])

            nc.sync.dma_start(RM[:], RM_d[:])
            nc.sync.dma_start(ZM[:], ZM_d[:])
            nc.scalar.dma_start(XF8[:, 1:T], XF8_d[:, 1:T])

            AGIN = [dram.tile([N, DH], F8, name=f"agin{i}") for i in range(3)]
            AGOUT = [dram.tile([2 * N, DH], F8, name=f"agout{i}") for i in range(3)]

            h = st.tile([128, NC * DH], FP32, tag="h", name="h_init")
            nc.gpsimd.memset(h[:], 0.0)

            def transpose_fm(src_nm, dst, sfx, dt=BF16):
                # src_nm [128, NC*64] node-major -> dst rows 0:64 feat-major
                # [64, N]; transpose two chunks per pass ([128,128] blocks:
                # rows 0:64 = even chunk's feats, 64:128 = odd chunk's)
                for q in range(2):
                    pt = psT.tile([128, 4 * 128], dt, tag="pt", name="pt" + sfx)
                    for ci in range(4):
                        c2 = q * 4 + ci
                        nc.tensor.transpose(
                            pt[:, ci * 128:(ci + 1) * 128],
                            src_nm[:, c2 * 2 * DH:(c2 + 1) * 2 * DH],
                            ident32[:] if dt is FP32 else ident[:])
                    dv = dst[0:DH, q * 1024:(q + 1) * 1024].rearrange(
                        "f (k two p) -> f k two p", two=2, p=128)
                    ptv = pt[:].rearrange("r (k p) -> r k p", p=128)
                    if q == 0:
                        nc.vector.tensor_copy(dv[:, :, 0], ptv[0:DH])
                        nc.scalar.activation(dv[:, :, 1], ptv[DH:128], AF.Copy)
                    else:
                        nc.scalar.activation(dv[:, :, 0], ptv[0:DH], AF.Copy)
                        nc.vector.tensor_copy(dv[:, :, 1], ptv[DH:128])

            for s in range(SLOTS):
                t = s % T
                sfx = f"_s{s}"

                # --- receive partner payload first (so the SP queue serves it
                # before this slot's 15us+ collective occupies the queue)
                xin = wk.tile([128, NC * DIN], BF16, tag="xin", name="xin" + sfx)
                xin_nm = xin[:].rearrange("p (c f) -> p c f", c=NC)
                if s >= SKEW:
                    recv8 = wk.tile([128, NC, DIN], F8, tag="rcv", name="rcv" + sfx)
                    nc.sync.dma_start(
                        recv8[:],
                        AGOUT[(s - 1) % 3][0:N, :].rearrange("(c p) f -> p c f", c=NC))
                    # xin = role * recv + X8[t]   (role: 0 on block-1 cores)
                    for hq in range(2):
                        cs_ = slice(hq * 8, (hq + 1) * 8)
                        nc.vector.scalar_tensor_tensor(
                            xin_nm[:, cs_], recv8[:, cs_], RM[:],
                            XF8[:, t, cs_],
                            op0=mybir.AluOpType.mult, op1=mybir.AluOpType.add)
                else:
                    nc.vector.tensor_copy(xin_nm, XF8[:, t])

                if s == SKEW:
                    hm = wk.tile([128, NC * DH], FP32, tag="h", name="hm")
                    nc.vector.tensor_mul(hm[:], h[:], ZM[:])
                    h = hm

                h_nm = h[:].rearrange("p (c f) -> p c f", c=NC)
                LH = wk.tile([128, NC, 128], F8, tag="LH", name="LH" + sfx)
                nc.vector.tensor_copy(LH[:, 0:8, DIN:128], h_nm[:, 0:8])
                nc.gpsimd.tensor_copy(LH[:, 8:NC, DIN:128], h_nm[:, 8:NC])

                if 1 <= s <= T:
                    agv = AGIN[s % 3][:].rearrange("(c p) f -> p c f", c=NC)
                    nc.sync.dma_start(agv[:, 0:8], LH[:, 0:8, DIN:128])
                    nc.sync.dma_start(agv[:, 8:NC], LH[:, 8:NC, DIN:128])
                    nc.gpsimd.collective_compute(
                        "AllGather", mybir.AluOpType.bypass,
                        ins=[AGIN[s % 3][:]], outs=[AGOUT[s % 3][:]],
                        replica_groups=RG)

                if s >= SKEW:
                    for hq in range(2):
                        cs_ = slice(hq * 8, (hq + 1) * 8)
                        nc.vector.scalar_tensor_tensor(
                            LH[:, cs_, 0:DIN], recv8[:, cs_], RM[:],
                            XF8[:, t, cs_],
                            op0=mybir.AluOpType.mult, op1=mybir.AluOpType.add)
                else:
                    nc.vector.tensor_copy(LH[:, 0:8, 0:DIN], XF8[:, t, 0:8])
                    nc.vector.tensor_copy(LH[:, 8:NC, 0:DIN], XF8[:, t, 8:NC])
                hbf = wk.tile([128, NC * DH], BF16, tag="hbf", name="hbf" + sfx)
                nc.scalar.activation(hbf[:], h[:], AF.Copy)

                XIN = wk.tile([DIN + 1, N], BF16, tag="XIN", name="XIN" + sfx)
                transpose_fm(xin, XIN, "x" + sfx)
                nc.vector.tensor_copy(XIN[DIN:DIN + 1, :], ones_bf[:])
                HT = wk.tile([DH, N], BF16, tag="HT", name="HT" + sfx)
                transpose_fm(hbf, HT, "h" + sfx)

                # --- packed [xin | h] diffusion + g gates, interleaved per
                # 512-col j-block so gates/sigmoid/rh trail the A stream.
                # Hop-1/2 results live as DoubleRow K-pairs in one fp8 tile,
                # scaled 2^-7 (psum carries 2^17 from A, stored s*2^10).
                P12 = w1.tile([128, 2, N], F8, tag="P12", name="P12" + sfx)
                g = w1.tile([128, NC * G], FP32, tag="g", name="g" + sfx)
                g_nm = g[:].rearrange("p (c f) -> p c f", c=NC)
                t1 = wk.tile([128, NC * DH], FP32, tag="t1", name="t1" + sfx)
                t1_nm = t1[:].rearrange("p (c f) -> p c f", c=NC)
                um1 = wk.tile([128, NC * DH], FP32, tag="um1", name="um1" + sfx)
                um1_nm = um1[:].rearrange("p (c f) -> p c f", c=NC)
                rh = wk.tile([128, NC * DH], FP32, tag="rh", name="rh" + sfx)
                rh_nm = rh[:].rearrange("p (c f) -> p c f", c=NC)
                rh8 = wk.tile([128, NC, DH], F8, tag="rh8", name="rh8" + sfx)
                for j in range(4):
                    js = slice(j * 512, (j + 1) * 512)
                    for hop, Asrc in enumerate((AT8, A2T8)):
                        pa = psA.tile([128, 512], FP32, tag="pa", name="pa" + sfx)
                        for kk in range(8):
                            nc.tensor.matmul(
                                pa[:],
                                LH[:, 2 * kk:2 * kk + 2, :],
                                Asrc[:, 2 * kk:2 * kk + 2, js],
                                start=(kk == 0), stop=(kk == 7),
                                perf_mode=DR)
                        if hop == 0:
                            nc.vector.tensor_scalar_mul(
                                P12[:, 0, js], pa[:], 2.0 ** -7)
                        else:
                            nc.scalar.activation(
                                P12[:, 1, js], pa[:], AF.Copy, scale=2.0 ** -7)
                    psg = psG.tile([128, 512], FP32, tag="psg", name="psg" + sfx)
                    for ci in range(4):
                        c = j * 4 + ci
                        o = psg[:, ci * G:(ci + 1) * G]
                        sl = slice(c * 128, (c + 1) * 128)
                        nc.tensor.matmul(o, XIN[:, sl], WG0X[:], start=True, stop=False)
                        nc.tensor.matmul(o, HT[:, sl], WG0H[:], start=False, stop=False)
                        nc.tensor.matmul(o, P12[:, :, sl], WG12[:], start=False,
                                         stop=True, perf_mode=DR)
                    nc.scalar.activation(g[:, js], psg[:], AF.Sigmoid,
                                         scale=2.0 ** -10)
                    cs = slice(j * 4, (j + 1) * 4)
                    nc.vector.tensor_mul(
                        rh_nm[:, cs], g_nm[:, cs, 0:DH], h_nm[:, cs])
                    nc.vector.tensor_copy(rh8[:, cs], rh_nm[:, cs])
                    # precompute u*h and (u-1) so the post-tanh update is
                    # only two ops: h_new = u*h - (u-1)*cc
                    nc.vector.tensor_mul(
                        t1_nm[:, cs], g_nm[:, cs, DH:G], h_nm[:, cs])
                    nc.vector.tensor_scalar_sub(
                        um1_nm[:, cs], g_nm[:, cs, DH:G], 1.0)
                rhbf = wk.tile([128, NC * DH], BF16, tag="rhbf", name="rhbf" + sfx)
                nc.scalar.activation(rhbf[:], rh[:], AF.Copy)
                RHT = wk.tile([DH, N], BF16, tag="RHT", name="RHT" + sfx)
                transpose_fm(rhbf, RHT, "r" + sfx)

                # --- c diffusion + c gates + h update, interleaved: 2 j-blocks
                # of SR per gate bank, then tanh + update per half
                SR12 = w1.tile([DH, 2, N], F8, tag="SR12", name="SR12" + sfx)
                cc = w1.tile([128, NC * DH], FP32, tag="cc", name="cc" + sfx)
                hmc = wk.tile([128, NC * DH], FP32, tag="hmc", name="hmc" + sfx)
                hmc_nm = hmc[:].rearrange("p (c f) -> p c f", c=NC)
                h_new = st.tile([128, NC * DH], FP32, tag="h", name="hn" + sfx)
                for j in range(4):
                    js = slice(j * 512, (j + 1) * 512)
                    for hop, Asrc in enumerate((AT8, A2T8)):
                        pc = psA.tile([DH, 512], FP32, tag="pa", name="pc" + sfx)
                        for kk in range(8):
                            nc.tensor.matmul(
                                pc[:],
                                rh8[:, 2 * kk:2 * kk + 2, :],
                                Asrc[:, 2 * kk:2 * kk + 2, js],
                                start=(kk == 0), stop=(kk == 7),
                                perf_mode=DR)
                        if hop == 0:
                            nc.vector.tensor_scalar_mul(
                                SR12[:, 0, js], pc[:], 2.0 ** -7)
                        else:
                            nc.scalar.activation(
                                SR12[:, 1, js], pc[:], AF.Copy,
                                scale=2.0 ** -7)
                    psc = psG.tile([128, 256], FP32, tag="psg", name="psc" + sfx)
                    for ci in range(4):
                        c = j * 4 + ci
                        o = psc[:, ci * DH:(ci + 1) * DH]
                        sl = slice(c * 128, (c + 1) * 128)
                        nc.tensor.matmul(o, XIN[:, sl], WC0X[:], start=True, stop=False)
                        nc.tensor.matmul(o, RHT[:, sl], WC0H[:], start=False, stop=False)
                        nc.tensor.matmul(o, P12[0:DIN, :, sl], WC12X[:],
                                         start=False, stop=False, perf_mode=DR)
                        nc.tensor.matmul(o, SR12[:, :, sl], WC12H[:],
                                         start=False, stop=True, perf_mode=DR)
                    qs_ = slice(j * 256, (j + 1) * 256)
                    nc.scalar.activation(cc[:, qs_], psc[:], AF.Tanh,
                                         scale=2.0 ** -10)
                    eng = nc.vector if j % 2 == 0 else nc.gpsimd
                    eng.tensor_mul(hmc[:, qs_], um1[:, qs_], cc[:, qs_])
                    eng.tensor_sub(h_new[:, qs_], t1[:, qs_], hmc[:, qs_])
                h = h_new

                if dbg_slot is not None and s == dbg_slot:
                    break

            nc.sync.dma_start(HOUT_d[:, 0:NC * DH // 2], h[:, 0:NC * DH // 2])
            nc.sync.dma_start(HOUT_d[:, NC * DH // 2:], h[:, NC * DH // 2:])

    nc.finalize()
    return nc


# ---------------------------------------------------------------------------
# host-side preparation and execution
# ---------------------------------------------------------------------------

def _prep_inputs(X, A_x, Wg, bg, Wc, bc):
    f32 = np.float32
    bf = ml_dtypes.bfloat16
    f8 = ml_dtypes.float8_e4m3
    A = A_x.astype(np.float64)
    A2 = A @ A

    # rhs layout [128, NC, N]: arr[p, k, n] = A[n, k*128+p] * SCALE
    def a_rhs(M):
        return np.ascontiguousarray(
            (M.T * SCALE).reshape(NC, 128, N).transpose(1, 0, 2)).astype(f8)
    AT8 = a_rhs(A)
    A2T8 = a_rhs(A2)

    def spec_norm(W):
        M = W.reshape(-1, W.shape[-1]).astype(np.float64)
        return W.astype(np.float64) / np.linalg.norm(M, ord=2)

    blk_w = []
    for blk in range(NBLK):
        Wg_n = spec_norm(Wg[blk])
        Wc_n = spec_norm(Wc[blk])
        # bf16 0-hop weights carry 2^10 (gate psum scale; activations apply
        # 2^-10). fp8 hop-1/2 weights are unscaled: inputs arrive as s*2^10.
        PS = 2.0 ** 10
        WG0X = np.zeros((DIN + 1, G), f32)
        WG0X[:DIN] = Wg_n[0][:DIN] * PS
        WG0X[DIN] = bg[blk] * PS
        WG0H = (Wg_n[0][DIN:] * PS).astype(f32)
        WG12 = np.stack([
            np.concatenate([Wg_n[1][:DIN], Wg_n[1][DIN:]], 0),
            np.concatenate([Wg_n[2][:DIN], Wg_n[2][DIN:]], 0)], 1)
        WC0X = np.zeros((DIN + 1, DH), f32)
        WC0X[:DIN] = Wc_n[0][:DIN] * PS
        WC0X[DIN] = bc[blk] * PS
        WC0H = (Wc_n[0][DIN:] * PS).astype(f32)
        WC12X = np.stack([Wc_n[1][:DIN], Wc_n[2][:DIN]], 1)
        WC12H = np.stack([Wc_n[1][DIN:], Wc_n[2][DIN:]], 1)
        blk_w.append({
            "WG0X": WG0X.astype(bf), "WG0H": WG0H.astype(bf),
            "WG12": WG12.astype(f8),
            "WC0X": WC0X.astype(bf), "WC0H": WC0H.astype(bf),
            "WC12X": WC12X.astype(f8), "WC12H": WC12H.astype(f8),
        })

    in_maps = []
    for core in range(8):
        b = core % B
        role = core // B  # 0 = block-1 runner, 1 = block-2 runner
        im = dict(blk_w[role])
        im["AT8"] = AT8
        im["A2T8"] = A2T8
        if role == 0:
            im["XF8"] = np.ascontiguousarray(
                X[b].reshape(T, NC, 128, DIN).transpose(2, 0, 1, 3)).astype(f8)
        else:
            im["XF8"] = np.zeros((128, T, NC, DIN), f8)
        im["RM"] = np.full((128, 1), float(role), f32)
        im["ZM"] = np.full((128, NC * DH), float(1 - role), f32)
        in_maps.append(im)
    return in_maps


_CACHED = {}


def _get_nc(trace_sim=False, dbg_slot=None):
    key = (trace_sim, dbg_slot)
    if key not in _CACHED:
        _CACHED[key] = build_kernel(trace_sim, dbg_slot)
    return _CACHED[key]


def run_on_device(inputs, dbg_slot=None):
    """Returns per-batch final h [B, N, DH] fp32 (block-2 cores' HOUT)."""
    nc = _get_nc(dbg_slot=dbg_slot)
    in_maps = _prep_inputs(inputs["X"], inputs["A_x"], inputs["Wg"], inputs["bg"],
                           inputs["Wc"], inputs["bc"])
    res = bass_utils.run_bass_kernel_spmd(nc, in_maps, core_ids=list(range(8)))
    results = res.results
    hs = []
    for b in range(B):
        hb = results[4 + b]["HOUT"].reshape(128, NC, DH).transpose(1, 0, 2)
        hs.append(hb.reshape(N, DH))
    return np.stack(hs), results


def kernel(**inputs):
    W_out = inputs["W_out"].astype(np.float64)
    b_out = inputs["b_out"].astype(np.float64)
    hs, _ = run_on_device(inputs)
    W_sn = W_out / np.linalg.norm(W_out)
    pred = hs.astype(np.float64) @ W_sn + b_out     # [B, N, 1]
    out = pred.squeeze(-1).mean()
    return np.float32(out)


if __name__ == "__main__":
    pass
